# revision 25
# baseline (speedup 1.0000x reference)
"""Trainium2 Bass kernel for a dense transformer decoder block.

Tensor-parallel over 8 NeuronCores, bf16 matmuls (fp32 accumulation):
  Phase A: heads sharded (2/core). norm+rope+QKV+causal attention over all
           tokens; per-batch AllToAll redistributes attention output to
           token owners (each core owns 256 tokens of each batch).
  Phase B: WO projection + residual into a persistent fp32 accumulator,
           then FFN rmsnorm -> y2^T.
  Phase C: FFN over the full hidden dim with streamed bf16 weights
           (no collectives); F2 accumulates into the fp32 accumulator.
"""
import sys

if '/opt/trn_rl_repo' not in sys.path:
    sys.path.insert(0, '/opt/trn_rl_repo')

import numpy as np
from contextlib import ExitStack

B, S, E, H, DH, F = 4, 2048, 2048, 16, 128, 8192
P = 128
NCORES = 8
HLOC = H // NCORES          # 2 heads per core
TOK = B * S                 # 8192 tokens
TSL = TOK // NCORES         # 1024 tokens per core (256 from each batch)
TB = TSL // B               # 256 tokens per (core, batch)
EC = E // P                 # 16 embedding chunks
FC = F // P                 # 64 FFN col chunks
FBLK = 4                    # outer F blocks
FCB = FC // FBLK            # 16 col chunks per block
EPS = 1e-5
ATB = 256                   # phase-A token block
NTB = S // ATB              # 8 blocks per batch

_CACHE = {}


def _build():
    import concourse.bacc as bacc
    import concourse.mybir as mybir
    import concourse.tile as tile
    import concourse.tile_utils as tile_utils
    from concourse.masks import make_identity

    tile_utils.max_sbuf_usage = 204 * 1024

    F32 = mybir.dt.float32
    F32R = mybir.dt.float32r
    BF = mybir.dt.bfloat16
    F8 = mybir.dt.float8e4
    DR = mybir.MatmulPerfMode.DoubleRow
    AF = mybir.ActivationFunctionType
    OP = mybir.AluOpType

    nc = bacc.Bacc(None, target_bir_lowering=False)
    names = {}

    with tile.TileContext(nc) as tc:
        with tc.tile_pool(name="dram", bufs=1, space="DRAM") as dram:
            # ---- external inputs ----
            x_in = dram.tile([TOK, E], BF, kind="ExternalInput")
            xsl_in = dram.tile([TSL, E], F32, kind="ExternalInput")
            wqkv_in = dram.tile([EC, P, 6 * P], F8, kind="ExternalInput")
            wo_in = dram.tile([H, P, E], F8, kind="ExternalInput")
            wg_in = dram.tile([FC, EC, P, P], BF, kind="ExternalInput")
            wl_in = dram.tile([FC, EC, P, P], BF, kind="ExternalInput")
            wout_in = dram.tile([FC, P, E], BF, kind="ExternalInput")
            cos_in = dram.tile([S, 64], BF, kind="ExternalInput")
            sin_in = dram.tile([S, 64], BF, kind="ExternalInput")
            mask_in = dram.tile([4, P, 512], F8, kind="ExternalInput")
            onec_in = dram.tile([P, 32], F8, kind="ExternalInput")
            oner_in = dram.tile([1, P], F32R, kind="ExternalInput")
            out_sl = dram.tile([TSL, E], F32, kind="ExternalOutput")
            names.update(
                x=x_in.name, xsl=xsl_in.name, wqkv=wqkv_in.name, wo=wo_in.name,
                wg=wg_in.name, wl=wl_in.name, wout=wout_in.name,
                cos=cos_in.name, sin=sin_in.name, mask=mask_in.name,
                onec=onec_in.name, oner=oner_in.name, out=out_sl.name)

            # ---- internal DRAM: per-batch AllToAll bounce ----
            a2a_in = [dram.tile([NCORES * HLOC * P, TB], BF, name=f"a2ai{b}")
                      for b in range(B)]
            a2a_out = [dram.tile([NCORES * HLOC * P, TB], BF,
                                 name=f"a2ao{b}")
                       for b in range(B)]

            RG = [list(range(NCORES))]

            with tc.tile_pool(name="cst", bufs=1) as cst, \
                 tc.tile_pool(name="acc_p", bufs=1) as acc_p, \
                 tc.tile_pool(name="y2T_p", bufs=1) as y2T_p:
                ident = cst.tile([P, P], BF)
                make_identity(nc, ident[:])
                eps_t = cst.tile([P, 1], F32)
                nc.gpsimd.memset(eps_t[:], EPS)
                neg1 = cst.tile([P, 1], F32)
                nc.gpsimd.memset(neg1[:], -1.0)
                c_wo = cst.tile([P, 1], F32)
                nc.gpsimd.memset(c_wo[:], 1.0 / 4096.0)

                # ================= phase A: norm+rope+QKV+attention =========
                stgA = ExitStack()
                wqkv_p = stgA.enter_context(tc.tile_pool(name="wqkv_p", bufs=1))
                tabs = stgA.enter_context(tc.tile_pool(name="tabs", bufs=1))
                ytb_p = stgA.enter_context(tc.tile_pool(name="ytb", bufs=2))
                qkvb_p = stgA.enter_context(tc.tile_pool(name="qkvb", bufs=1))
                st_sb = stgA.enter_context(tc.tile_pool(name="st_sb", bufs=2))
                scr_p = stgA.enter_context(tc.tile_pool(name="scr_p", bufs=1))
                st_ps = stgA.enter_context(tc.tile_pool(name="st_ps", bufs=1, space="PSUM"))
                qkv_ps = stgA.enter_context(tc.tile_pool(name="qkv_ps", bufs=1, space="PSUM"))
                at_s_ps = stgA.enter_context(tc.tile_pool(name="at_s_ps", bufs=2, space="PSUM"))
                at_o_ps = stgA.enter_context(tc.tile_pool(name="at_o_ps", bufs=1, space="PSUM"))
                at_db_ps = stgA.enter_context(tc.tile_pool(name="at_db_ps", bufs=1, space="PSUM"))
                at_sb = stgA.enter_context(tc.tile_pool(name="at_sb", bufs=2))

                wqkv_sb = wqkv_p.tile([P, EC * 6 * P], F8)
                nc.sync.dma_start(
                    out=wqkv_sb[:].rearrange("p (e c) -> p e c", e=EC),
                    in_=wqkv_in[:].rearrange("e p c -> p e c"))
                ones_col = tabs.tile([P, 32], F8)
                nc.sync.dma_start(out=ones_col[:], in_=onec_in[:])
                ones_row = tabs.tile([1, P], F32R)
                nc.sync.dma_start(out=ones_row[:], in_=oner_in[:])
                masks = tabs.tile([P, 4 * 512], F8)
                nc.sync.dma_start(
                    out=masks[:].rearrange("p (m w) -> p m w", m=4),
                    in_=mask_in[:].rearrange("m p w -> p m w"))
                # rope tables [s, j<64] -> sbuf [s%128, (srange, j)]
                cos_all = tabs.tile([P, EC * 64], BF)
                sin_all = tabs.tile([P, EC * 64], BF)
                nc.sync.dma_start(
                    out=cos_all[:].rearrange("p (r j) -> p r j", r=EC),
                    in_=cos_in[:].rearrange("(r p) j -> p r j", p=P))
                nc.sync.dma_start(
                    out=sin_all[:].rearrange("p (r j) -> p r j", r=EC),
                    in_=sin_in[:].rearrange("(r p) j -> p r j", p=P))

                for b in range(B):
                    qt_b = qkvb_p.tile([P, HLOC * S], BF, tag="qt")
                    kt_b = qkvb_p.tile([P, HLOC * S], BF, tag="kt")
                    v_b = qkvb_p.tile([P, HLOC * S], F8, tag="vb")
                    for tb in range(NTB):    # 256-token blocks
                        yT = ytb_p.tile([P, EC * ATB], F8, tag="yT")
                        for tt in range(ATB // P):  # 128-token tiles
                            row = S * b + ATB * tb + P * tt
                            sr = 2 * tb + tt  # position block index
                            x_t = st_sb.tile([P, E], BF, tag="x")
                            nc.sync.dma_start(out=x_t[:], in_=x_in[row:row + P, :])
                            scr = scr_p.tile([P, E], BF, tag="scr")
                            ssq = st_sb.tile([P, 1], F32, tag="ssq")
                            nc.scalar.activation(scr[:], x_t[:], AF.Square,
                                                 accum_out=ssq[:])
                            lg = st_sb.tile([P, 1], F32, tag="sq")
                            nc.scalar.activation(lg[:], ssq[:], AF.Ln,
                                                 scale=1.0 / E, bias=eps_t[:])
                            s_t = st_sb.tile([P, 1], F32, tag="s")
                            nc.scalar.activation(s_t[:], lg[:], AF.Exp,
                                                 scale=-0.5)
                            # rope with rmsnorm scale folded in (DVE bf16):
                            #   y1 = (x1*s)*cos - (x2*s)*sin
                            #   y2 = (x2*s)*cos + (x1*s)*sin
                            y_t = st_sb.tile([P, E], BF, tag="y")
                            t1 = st_sb.tile([P, E], BF, tag="t1")
                            xr = x_t[:].rearrange("p (c two h) -> p c two h", two=2, h=64)
                            yr = y_t[:].rearrange("p (c two h) -> p c two h", two=2, h=64)
                            tr = t1[:].rearrange("p (c two h) -> p c two h", two=2, h=64)
                            cb = cos_all[:, 64 * sr:64 * (sr + 1)].rearrange(
                                "p (o j) -> p o j", o=1).broadcast_to([P, EC, 64])
                            sb_ = sin_all[:, 64 * sr:64 * (sr + 1)].rearrange(
                                "p (o j) -> p o j", o=1).broadcast_to([P, EC, 64])
                            nc.vector.scalar_tensor_tensor(
                                out=tr[:, :, 0], in0=xr[:, :, 1], scalar=s_t[:],
                                in1=sb_, op0=OP.mult, op1=OP.mult)
                            nc.vector.scalar_tensor_tensor(
                                out=yr[:, :, 0], in0=xr[:, :, 0], scalar=s_t[:],
                                in1=cb, op0=OP.mult, op1=OP.mult)
                            nc.vector.tensor_tensor(
                                out=yr[:, :, 0], in0=yr[:, :, 0], in1=tr[:, :, 0],
                                op=OP.subtract)
                            nc.vector.scalar_tensor_tensor(
                                out=tr[:, :, 1], in0=xr[:, :, 0], scalar=s_t[:],
                                in1=sb_, op0=OP.mult, op1=OP.mult)
                            nc.vector.scalar_tensor_tensor(
                                out=yr[:, :, 1], in0=xr[:, :, 1], scalar=s_t[:],
                                in1=cb, op0=OP.mult, op1=OP.mult)
                            nc.vector.tensor_tensor(
                                out=yr[:, :, 1], in0=yr[:, :, 1], in1=tr[:, :, 1],
                                op=OP.add)
                            # transpose 16 chunks -> yT (4 psum groups of 4)
                            for gch in range(4):
                                tps = st_ps.tile([P, 512], BF, tag="tps")
                                for c4 in range(4):
                                    c = 4 * gch + c4
                                    nc.tensor.transpose(
                                        tps[:, P * c4:P * (c4 + 1)],
                                        y_t[:, P * c:P * (c + 1)], ident[:])
                                yT_dst = yT[:].rearrange("p (c t) -> p c t", t=ATB)
                                if gch < 2:
                                    nc.scalar.activation(
                                        yT_dst[:, 4 * gch:4 * gch + 4, P * tt:P * (tt + 1)],
                                        tps[:].rearrange("p (c t) -> p c t", t=P),
                                        AF.Copy, scale=16.0)
                                else:
                                    nc.vector.tensor_scalar_mul(
                                        yT_dst[:, 4 * gch:4 * gch + 4, P * tt:P * (tt + 1)],
                                        tps[:].rearrange("p (c t) -> p c t", t=P),
                                        16.0)
                        # QKV matmuls (6 groups of 128 cols)
                        for half in range(2):
                            for g3 in range(3):
                                g = 3 * half + g3
                                pq = qkv_ps.tile([P, ATB], F32, tag=f"pq{g3}")
                                wv_v = wqkv_sb[:].rearrange(
                                    "p (e c) -> p e c", e=EC)
                                yT_v = yT[:].rearrange("p (e t) -> p e t", e=EC)
                                for ep in range(EC // 2):
                                    nc.tensor.matmul(
                                        pq[:],
                                        wv_v[:, 2 * ep:2 * ep + 2, P * g:P * (g + 1)],
                                        yT_v[:, 2 * ep:2 * ep + 2, :],
                                        start=(ep == 0), stop=(ep == EC // 2 - 1),
                                        perf_mode=DR)
                                h = g % 2
                                col = S * h + ATB * tb
                                if g < 2:      # Q heads (scale folded host-side)
                                    nc.scalar.activation(qt_b[:, col:col + ATB],
                                                         pq[:], AF.Copy,
                                                         scale=1.0 / 8192.0)
                                elif g < 4:    # K heads
                                    nc.scalar.activation(kt_b[:, col:col + ATB],
                                                         pq[:], AF.Copy,
                                                         scale=1.0 / 8192.0)
                                else:          # V heads -> transpose to [tok, DH]
                                    vt_tmp = st_sb.tile([P, ATB], BF, tag="vt")
                                    nc.scalar.activation(vt_tmp[:], pq[:], AF.Copy,
                                                         scale=1.0 / 512.0)
                                    tpv = st_ps.tile([P, 512], BF, tag="tps")
                                    for j in range(ATB // P):
                                        nc.tensor.transpose(
                                            tpv[:, P * j:P * (j + 1)],
                                            vt_tmp[:, P * j:P * (j + 1)], ident[:])
                                    nc.vector.tensor_copy(v_b[:, col:col + ATB],
                                                          tpv[:, 0:ATB])
                    # ---- attention for batch b, both local heads ----
                    for h in range(HLOC):
                        hs = S * h
                        for qb in range(4):
                            nk = 4 * qb + 4
                            ps_o = at_o_ps.tile([P, 512], F32, tag="pso")
                            ps_d = at_db_ps.tile([1, 512], F32, tag="db")
                            nkp = nk // 2
                            for ktp in range(nkp):
                                probs = at_sb.tile([P, 1024], F8, tag="probs")
                                for sub in range(2):
                                    kt = 2 * ktp + sub
                                    ps_s = at_s_ps.tile([P, 512], F32, tag="pss")
                                    nc.tensor.matmul(
                                        ps_s[:],
                                        kt_b[:, hs + P * kt: hs + P * (kt + 1)],
                                        qt_b[:, hs + 512 * qb: hs + 512 * (qb + 1)],
                                        start=True, stop=True)
                                    pr = probs[:, 512 * sub:512 * (sub + 1)]
                                    nc.scalar.activation(pr, ps_s[:], AF.Exp,
                                                         bias=neg1[:])
                                    if kt >= 4 * qb:
                                        m = kt - 4 * qb
                                        nc.vector.tensor_tensor(
                                            out=pr, in0=pr,
                                            in1=masks[:, 512 * m:512 * (m + 1)],
                                            op=OP.mult)
                                pr2 = probs[:].rearrange(
                                    "p (two t) -> p two t", two=2)
                                nc.tensor.matmul(
                                    ps_o[:],
                                    v_b[:, hs + 2 * P * ktp: hs + 2 * P * (ktp + 1)].rearrange(
                                        "p (two t) -> p two t", two=2),
                                    pr2,
                                    start=(ktp == 0), stop=(ktp == nkp - 1),
                                    skip_group_check=True, perf_mode=DR)
                                nc.tensor.matmul(
                                    ps_d[:],
                                    ones_col[:].rearrange(
                                        "p (two t) -> p two t", two=2)[:, :, 0:1],
                                    pr2,
                                    start=(ktp == 0), stop=(ktp == nkp - 1),
                                    skip_group_check=True, perf_mode=DR)
                            rd = scr_p.tile([1, 512], F32R, tag="rd")
                            with nc.allow_low_precision(reason="softmax denom"):
                                nc.vector.reciprocal(rd[:], ps_d[:])
                            ps_b = at_db_ps.tile([P, 512], F32, tag="db")
                            nc.tensor.matmul(ps_b[:], ones_row[:], rd[:],
                                             start=True, stop=True)
                            osb = at_sb.tile([P, 512], BF, tag="osb")
                            nc.scalar.copy(osb[:], ps_o[:])
                            ot = at_sb.tile([P, 512], BF, tag="ot")
                            nc.vector.tensor_tensor(out=ot[:], in0=osb[:],
                                                    in1=ps_b[:], op=OP.mult)
                            # tokens 512*qb..512*qb+512 of batch b ->
                            # dest cores 2qb (first 256) and 2qb+1
                            a2a_v = a2a_in[b][:].rearrange(
                                "(d r) t -> d r t", d=NCORES)
                            for m in range(2):
                                nc.sync.dma_start(
                                    out=a2a_v[2 * qb + m, P * h:P * (h + 1), :],
                                    in_=ot[:, TB * m:TB * (m + 1)])
                    nc.gpsimd.collective_compute(
                        "AllToAll", OP.bypass, replica_groups=RG,
                        ins=[a2a_in[b][:]], outs=[a2a_out[b][:]])
                stgA.close()

                # ================= phase B: WO + residual + FFN norm ========
                # fp32 accumulator tiles hold x + attn@wo, later += FFN out.
                acc = []
                for t8 in range(8):
                    a = acc_p.tile([P, E], F32, name=f"acc{t8}")
                    nc.gpsimd.dma_start(out=a[:], in_=xsl_in[P * t8:P * (t8 + 1), :])
                    acc.append(a)
                y2T = y2T_p.tile([P, EC * TSL], BF)

                phB = ExitStack()
                ot_p = phB.enter_context(tc.tile_pool(name="ot_p", bufs=1))
                wo_p = phB.enter_context(tc.tile_pool(name="wo_p", bufs=2))
                wo_sb = phB.enter_context(tc.tile_pool(name="wo_sb", bufs=2))
                wo_ps = phB.enter_context(tc.tile_pool(name="wo_ps", bufs=2, space="PSUM"))
                wo_tps = phB.enter_context(tc.tile_pool(name="wo_tps", bufs=2, space="PSUM"))

                ot_sb = []
                for b in range(B):
                    o = ot_p.tile([P, H * TB], BF, name=f"otb{b}")
                    nc.gpsimd.dma_start(
                        out=o[:].rearrange("p (hc t) -> p hc t", hc=H),
                        in_=a2a_out[b][:].rearrange("(hc p) t -> p hc t", p=P))
                    o8 = ot_p.tile([P, H * TB], F8, name=f"otb8{b}")
                    nc.scalar.copy(o8[:], o[:])
                    ot_sb.append(o8)

                for ecol in range(4):
                    wo_c = wo_p.tile([P, H * 512], F8, tag="woc")
                    nc.gpsimd.dma_start(
                        out=wo_c[:].rearrange("p (hc w) -> p hc w", hc=H),
                        in_=wo_in[:, :, 512 * ecol:512 * (ecol + 1)].rearrange(
                            "hc p w -> p hc w"))
                    wo_v = wo_c[:].rearrange("p (hc w) -> p hc w", hc=H)
                    for t8 in range(8):
                        b, tt = t8 // 2, t8 % 2
                        ot_v = ot_sb[b][:].rearrange("p (hc t) -> p hc t", hc=H)
                        ps = wo_ps.tile([P, 512], F32, tag="ps")
                        for hp in range(H // 2):
                            nc.tensor.matmul(
                                ps[:],
                                ot_v[:, 2 * hp:2 * hp + 2, P * tt:P * (tt + 1)],
                                wo_v[:, 2 * hp:2 * hp + 2, :],
                                start=(hp == 0), stop=(hp == H // 2 - 1),
                                perf_mode=DR)
                        nc.vector.scalar_tensor_tensor(
                            out=acc[t8][:, 512 * ecol:512 * (ecol + 1)],
                            in0=ps[:], scalar=c_wo[:],
                            in1=acc[t8][:, 512 * ecol:512 * (ecol + 1)],
                            op0=OP.mult, op1=OP.add)
                # FFN rmsnorm + transpose -> y2T [e, tok]
                for t8 in range(8):
                    scr2 = wo_sb.tile([P, E], BF, tag="scr2")
                    ssq2 = wo_sb.tile([P, 1], F32, tag="ssq2")
                    nc.scalar.activation(scr2[:], acc[t8][:], AF.Square,
                                         accum_out=ssq2[:])
                    lg2 = wo_sb.tile([P, 1], F32, tag="sq2")
                    nc.scalar.activation(lg2[:], ssq2[:], AF.Ln,
                                         scale=1.0 / E, bias=eps_t[:])
                    s2 = wo_sb.tile([P, 1], F32, tag="s2")
                    nc.scalar.activation(s2[:], lg2[:], AF.Exp,
                                         scale=-0.5)
                    y2_t = wo_sb.tile([P, E], BF, tag="y2")
                    nc.scalar.activation(y2_t[:], acc[t8][:], AF.Copy,
                                         scale=s2[:])
                    y2T_v = y2T[:].rearrange("p (c t) -> p c t", t=TSL)
                    for gch in range(4):
                        tps = wo_tps.tile([P, 512], BF, tag="tps")
                        for c4 in range(4):
                            c = 4 * gch + c4
                            nc.tensor.transpose(
                                tps[:, P * c4:P * (c4 + 1)],
                                y2_t[:, P * c:P * (c + 1)], ident[:])
                        nc.vector.tensor_copy(
                            y2T_v[:, 4 * gch:4 * gch + 4, P * t8:P * (t8 + 1)],
                            tps[:].rearrange("p (c t) -> p c t", t=P))
                phB.close()

                # ================= phase C: FFN, streamed full weights ======
                phC = ExitStack()
                wf_p = phC.enter_context(tc.tile_pool(name="wf_p", bufs=2))
                hT_p = phC.enter_context(tc.tile_pool(name="hT_p", bufs=1))
                f1_sb = phC.enter_context(tc.tile_pool(name="f1_sb", bufs=2))
                f1_gps = phC.enter_context(tc.tile_pool(name="f1_gps", bufs=2, space="PSUM"))
                f1_lps = phC.enter_context(tc.tile_pool(name="f1_lps", bufs=2, space="PSUM"))
                f2_ps = phC.enter_context(tc.tile_pool(name="f2_ps", bufs=2, space="PSUM"))

                for k in range(FBLK):
                    hT = hT_p.tile([P, FCB * TSL], BF, tag="hT")
                    for fc in range(FCB):
                        wgs = wf_p.tile([P, EC * P], BF, tag="wg")
                        nc.sync.dma_start(
                            out=wgs[:].rearrange("p (e c) -> p e c", e=EC),
                            in_=wg_in[FCB * k + fc].rearrange("e p c -> p e c"))
                        wls = wf_p.tile([P, EC * P], BF, tag="wl")
                        nc.sync.dma_start(
                            out=wls[:].rearrange("p (e c) -> p e c", e=EC),
                            in_=wl_in[FCB * k + fc].rearrange("e p c -> p e c"))
                        for g2 in range(2):   # 512-token groups
                            psg = f1_gps.tile([P, 512], F32, tag="psg")
                            psl = f1_lps.tile([P, 512], F32, tag="psl")
                            for ec in range(EC):
                                mv = y2T[:, TSL * ec + 512 * g2:
                                         TSL * ec + 512 * (g2 + 1)]
                                nc.tensor.matmul(
                                    psg[:], wgs[:, P * ec:P * (ec + 1)], mv,
                                    start=(ec == 0), stop=(ec == EC - 1),
                                    skip_group_check=True)
                                nc.tensor.matmul(
                                    psl[:], wls[:, P * ec:P * (ec + 1)], mv,
                                    start=(ec == 0), stop=(ec == EC - 1),
                                    skip_group_check=True)
                            tmp_g = f1_sb.tile([P, 512], BF, tag="tmpg")
                            nc.scalar.activation(tmp_g[:], psg[:], AF.Gelu)
                            nc.vector.tensor_tensor(
                                out=hT[:, TSL * fc + 512 * g2:
                                       TSL * fc + 512 * (g2 + 1)],
                                in0=tmp_g[:], in1=psl[:], op=OP.mult)
                    # F2: out partial [tok, E] accumulated into acc
                    for ecol in range(4):
                        wos = wf_p.tile([P, FCB * 512], BF, tag="wos")
                        nc.sync.dma_start(
                            out=wos[:].rearrange("p (f w) -> p f w", f=FCB),
                            in_=wout_in[FCB * k:FCB * (k + 1), :,
                                        512 * ecol:512 * (ecol + 1)].rearrange(
                                "f p w -> p f w"))
                        for t8 in range(8):
                            ps2 = f2_ps.tile([P, 512], F32, tag="ps2")
                            for fs in range(FCB):
                                nc.tensor.matmul(
                                    ps2[:],
                                    hT[:, TSL * fs + P * t8: TSL * fs + P * (t8 + 1)],
                                    wos[:, 512 * fs:512 * (fs + 1)],
                                    start=(fs == 0), stop=(fs == FCB - 1))
                            nc.vector.tensor_tensor(
                                out=acc[t8][:, 512 * ecol:512 * (ecol + 1)],
                                in0=ps2[:],
                                in1=acc[t8][:, 512 * ecol:512 * (ecol + 1)],
                                op=OP.add)
                            if k == FBLK - 1 and ecol == 3:
                                nc.sync.dma_start(
                                    out=out_sl[P * t8:P * (t8 + 1), :],
                                    in_=acc[t8][:])
                phC.close()
    nc.compile()
    return nc, names


def _prep_inputs_shared(inputs):
    """Host-side prep of tensors identical on every core."""
    import ml_dtypes
    BF = ml_dtypes.bfloat16
    x = np.ascontiguousarray(
        np.asarray(inputs["inputs"], np.float32).reshape(TOK, E))
    wo = np.asarray(inputs["wo"], np.float32)
    w_gate = np.asarray(inputs["w_gate"], np.float32)
    w_lin = np.asarray(inputs["w_lin"], np.float32)
    w_out = np.asarray(inputs["w_out"], np.float32)
    gamma_attn = np.asarray(inputs["gamma_attn"], np.float32)
    gamma_ffn = np.asarray(inputs["gamma_ffn"], np.float32)
    positions = np.asarray(inputs["positions"])

    F8 = ml_dtypes.float8_e4m3
    x_bf = np.ascontiguousarray(x.astype(BF))
    wo_r = np.ascontiguousarray((wo.reshape(H, P, E) * 256.0).astype(F8))
    # [E, F] -> [FC, EC, P, P] (fcol-major strips)
    def _gl(w):
        w = (w * gamma_ffn[:, None]).astype(BF)
        return np.ascontiguousarray(
            w.reshape(EC, P, FC, P).transpose(2, 0, 1, 3))
    wg_t = _gl(w_gate)
    wl_t = _gl(w_lin)
    wout_t = np.ascontiguousarray(w_out.reshape(FC, P, E).astype(BF))

    assert np.all(gamma_attn == gamma_attn[0]), \
        "non-uniform gamma_attn needs full-width rope tables"
    half = DH // 2
    inv_freq = (1.0 / (10000.0 ** (np.arange(half, dtype=np.float32) / half))
                ).astype(np.float32)
    ang = positions.astype(np.float32)[:, None] * inv_freq[None, :]
    g0 = float(gamma_attn[0])
    cos = (np.cos(ang) * g0).astype(BF)
    sin = (np.sin(ang) * g0).astype(BF)

    k_i = np.arange(P)[:, None]
    q_i = np.arange(512)[None, :]
    msk = np.stack([(P * m + k_i <= q_i) for m in range(4)]).astype(F8)

    return {
        "x": x_bf, "wo": wo_r, "wg": wg_t, "wl": wl_t, "wout": wout_t,
        "cos": cos, "sin": sin, "mask": msk,
        "onec": np.ones((P, 32), F8),
        "oner": np.ones((1, P), np.float32),
    }, x


def _prep_inputs_core(inputs, x_f32, r):
    """Per-core tensors: head-sharded QKV weights + owned-token x slice."""
    import ml_dtypes
    F8 = ml_dtypes.float8_e4m3
    wq = np.asarray(inputs["wq"], np.float32) / np.sqrt(np.float32(DH))
    wk = np.asarray(inputs["wk"], np.float32)
    wv = np.asarray(inputs["wv"], np.float32)
    h0 = HLOC * r

    def _slice_qkv(w):   # [E, H, DH] -> [EC, P, HLOC*DH]
        return w[:, h0:h0 + HLOC, :].reshape(EC, P, HLOC * DH)

    wqkv = (np.concatenate([_slice_qkv(wq), _slice_qkv(wk), _slice_qkv(wv)],
                           axis=2) * 512.0).astype(F8)
    # tokens owned by core r: 256 from each batch
    xsl = np.ascontiguousarray(
        x_f32.reshape(B, NCORES, TB, E)[:, r].reshape(TSL, E))
    return {"wqkv": np.ascontiguousarray(wqkv), "xsl": xsl}


def _run(inputs, trace=False):
    from concourse.bass_utils import run_bass_kernel_spmd

    if "nc" not in _CACHE:
        _CACHE["nc"], _CACHE["names"] = _build()
    nc, names = _CACHE["nc"], _CACHE["names"]

    shared, x_f32 = _prep_inputs_shared(inputs)
    in_maps = []
    for r in range(NCORES):
        prep = dict(shared)
        prep.update(_prep_inputs_core(inputs, x_f32, r))
        in_maps.append({names[k]: v for k, v in prep.items()})

    res = run_bass_kernel_spmd(nc, in_maps, core_ids=list(range(NCORES)),
                               trace=trace)
    out = np.empty((B, NCORES, TB, E), np.float32)
    for r in range(NCORES):
        out[:, r] = res.results[r][names["out"]].reshape(B, TB, E)
    return out.reshape(B, S, E), res


def kernel(**inputs) -> np.ndarray:
    return _run(inputs)[0]


# revision 26
# speedup vs baseline: 1.0711x; 1.0711x over previous
"""Trainium2 Bass kernel for a dense transformer decoder block.

Tensor-parallel over 8 NeuronCores, bf16 matmuls (fp32 accumulation):
  Phase A: heads sharded (2/core). norm+rope+QKV+causal attention over all
           tokens; per-batch AllToAll redistributes attention output to
           token owners (each core owns 256 tokens of each batch).
  Phase B: WO projection + residual into a persistent fp32 accumulator,
           then FFN rmsnorm -> y2^T.
  Phase C: FFN over the full hidden dim with streamed bf16 weights
           (no collectives); F2 accumulates into the fp32 accumulator.
"""
import sys

if '/opt/trn_rl_repo' not in sys.path:
    sys.path.insert(0, '/opt/trn_rl_repo')

import numpy as np
from contextlib import ExitStack

B, S, E, H, DH, F = 4, 2048, 2048, 16, 128, 8192
P = 128
NCORES = 8
HLOC = H // NCORES          # 2 heads per core
TOK = B * S                 # 8192 tokens
TSL = TOK // NCORES         # 1024 tokens per core (256 from each batch)
TB = TSL // B               # 256 tokens per (core, batch)
EC = E // P                 # 16 embedding chunks
FC = F // P                 # 64 FFN col chunks
FBLK = 4                    # outer F blocks
FCB = FC // FBLK            # 16 col chunks per block
EPS = 1e-5
ATB = 256                   # phase-A token block
NTB = S // ATB              # 8 blocks per batch

_CACHE = {}


def _build():
    import concourse.bacc as bacc
    import concourse.mybir as mybir
    import concourse.tile as tile
    import concourse.tile_utils as tile_utils
    from concourse.masks import make_identity

    tile_utils.max_sbuf_usage = 204 * 1024

    F32 = mybir.dt.float32
    F32R = mybir.dt.float32r
    BF = mybir.dt.bfloat16
    F8 = mybir.dt.float8e4
    DR = mybir.MatmulPerfMode.DoubleRow
    AF = mybir.ActivationFunctionType
    OP = mybir.AluOpType

    nc = bacc.Bacc(None, target_bir_lowering=False)
    names = {}

    with tile.TileContext(nc) as tc:
        with tc.tile_pool(name="dram", bufs=1, space="DRAM") as dram:
            # ---- external inputs ----
            x_in = dram.tile([TOK, E], BF, kind="ExternalInput")
            xsl_in = dram.tile([TSL, E], F32, kind="ExternalInput")
            wqkv_in = dram.tile([EC, P, 6 * P], F8, kind="ExternalInput")
            wo_in = dram.tile([H, P, E], F8, kind="ExternalInput")
            wg_in = dram.tile([FC, EC, P, P], BF, kind="ExternalInput")
            wl_in = dram.tile([FC, EC, P, P], BF, kind="ExternalInput")
            wout_in = dram.tile([FC, P, E], BF, kind="ExternalInput")
            cos_in = dram.tile([S, 64], BF, kind="ExternalInput")
            sin_in = dram.tile([S, 64], BF, kind="ExternalInput")
            mask_in = dram.tile([4, P, 512], F8, kind="ExternalInput")
            onec_in = dram.tile([P, 32], F8, kind="ExternalInput")
            oner_in = dram.tile([1, P], F32R, kind="ExternalInput")
            out_sl = dram.tile([TSL, E], F32, kind="ExternalOutput")
            names.update(
                x=x_in.name, xsl=xsl_in.name, wqkv=wqkv_in.name, wo=wo_in.name,
                wg=wg_in.name, wl=wl_in.name, wout=wout_in.name,
                cos=cos_in.name, sin=sin_in.name, mask=mask_in.name,
                onec=onec_in.name, oner=oner_in.name, out=out_sl.name)

            # ---- internal DRAM: per-batch AllToAll bounce ----
            a2a_in = [dram.tile([NCORES * HLOC * P, TB], BF, name=f"a2ai{b}")
                      for b in range(B)]
            a2a_out = [dram.tile([NCORES * HLOC * P, TB], BF,
                                 name=f"a2ao{b}")
                       for b in range(B)]

            RG = [list(range(NCORES))]

            with tc.tile_pool(name="cst", bufs=1) as cst, \
                 tc.tile_pool(name="acc_p", bufs=1) as acc_p, \
                 tc.tile_pool(name="y2T_p", bufs=1) as y2T_p:
                ident = cst.tile([P, P], BF)
                make_identity(nc, ident[:])
                eps_t = cst.tile([P, 1], F32)
                nc.gpsimd.memset(eps_t[:], EPS)
                neg1 = cst.tile([P, 1], F32)
                nc.gpsimd.memset(neg1[:], -1.0)
                c_wo = cst.tile([P, 1], F32)
                nc.gpsimd.memset(c_wo[:], 1.0 / 4096.0)

                # ================= phase A: norm+rope+QKV+attention =========
                stgA = ExitStack()
                wqkv_p = stgA.enter_context(tc.tile_pool(name="wqkv_p", bufs=1))
                tabs = stgA.enter_context(tc.tile_pool(name="tabs", bufs=1))
                ytb_p = stgA.enter_context(tc.tile_pool(name="ytb", bufs=2))
                qkvb_p = stgA.enter_context(tc.tile_pool(name="qkvb", bufs=1))
                st_sb = stgA.enter_context(tc.tile_pool(name="st_sb", bufs=2))
                scr_p = stgA.enter_context(tc.tile_pool(name="scr_p", bufs=1))
                st_ps = stgA.enter_context(tc.tile_pool(name="st_ps", bufs=2, space="PSUM"))
                qkv_ps = stgA.enter_context(tc.tile_pool(name="qkv_ps", bufs=1, space="PSUM"))
                at_s_ps = stgA.enter_context(tc.tile_pool(name="at_s_ps", bufs=2, space="PSUM"))
                at_o_ps = stgA.enter_context(tc.tile_pool(name="at_o_ps", bufs=1, space="PSUM"))
                at_db_ps = stgA.enter_context(tc.tile_pool(name="at_db_ps", bufs=1, space="PSUM"))
                at_sb = stgA.enter_context(tc.tile_pool(name="at_sb", bufs=2))

                wqkv_sb = wqkv_p.tile([P, EC * 6 * P], F8)
                nc.sync.dma_start(
                    out=wqkv_sb[:].rearrange("p (e c) -> p e c", e=EC),
                    in_=wqkv_in[:].rearrange("e p c -> p e c"))
                ones_col = tabs.tile([P, 32], F8)
                nc.sync.dma_start(out=ones_col[:], in_=onec_in[:])
                ones_row = tabs.tile([1, P], F32R)
                nc.sync.dma_start(out=ones_row[:], in_=oner_in[:])
                masks = tabs.tile([P, 4 * 512], F8)
                nc.sync.dma_start(
                    out=masks[:].rearrange("p (m w) -> p m w", m=4),
                    in_=mask_in[:].rearrange("m p w -> p m w"))
                # rope tables [s, j<64] -> sbuf [s%128, (srange, j)]
                cos_all = tabs.tile([P, EC * 64], BF)
                sin_all = tabs.tile([P, EC * 64], BF)
                nc.sync.dma_start(
                    out=cos_all[:].rearrange("p (r j) -> p r j", r=EC),
                    in_=cos_in[:].rearrange("(r p) j -> p r j", p=P))
                nc.sync.dma_start(
                    out=sin_all[:].rearrange("p (r j) -> p r j", r=EC),
                    in_=sin_in[:].rearrange("(r p) j -> p r j", p=P))

                for b in range(B):
                    qt_b = qkvb_p.tile([P, HLOC * S], BF, tag="qt")
                    kt_b = qkvb_p.tile([P, HLOC * S], BF, tag="kt")
                    v_b = qkvb_p.tile([P, HLOC * S], F8, tag="vb")
                    for tb in range(NTB):    # 256-token blocks
                        yT = ytb_p.tile([P, EC * ATB], F8, tag="yT")
                        for tt in range(ATB // P):  # 128-token tiles
                            row = S * b + ATB * tb + P * tt
                            sr = 2 * tb + tt  # position block index
                            x_t = st_sb.tile([P, E], BF, tag="x")
                            nc.sync.dma_start(out=x_t[:], in_=x_in[row:row + P, :])
                            scr = scr_p.tile([P, E], BF, tag="scr")
                            ssq = st_sb.tile([P, 1], F32, tag="ssq")
                            nc.scalar.activation(scr[:], x_t[:], AF.Square,
                                                 accum_out=ssq[:])
                            sq = st_sb.tile([P, 1], F32, tag="sq")
                            nc.scalar.activation(sq[:], ssq[:], AF.Sqrt,
                                                 scale=1.0 / E, bias=eps_t[:])
                            s_t = st_sb.tile([P, 1], F32, tag="s")
                            nc.vector.reciprocal(s_t[:], sq[:])
                            # rope with rmsnorm scale folded in (DVE bf16):
                            #   y1 = (x1*s)*cos - (x2*s)*sin
                            #   y2 = (x2*s)*cos + (x1*s)*sin
                            y_t = st_sb.tile([P, E], BF, tag="y")
                            t1 = st_sb.tile([P, E], BF, tag="t1")
                            xr = x_t[:].rearrange("p (c two h) -> p c two h", two=2, h=64)
                            yr = y_t[:].rearrange("p (c two h) -> p c two h", two=2, h=64)
                            tr = t1[:].rearrange("p (c two h) -> p c two h", two=2, h=64)
                            cb = cos_all[:, 64 * sr:64 * (sr + 1)].rearrange(
                                "p (o j) -> p o j", o=1).broadcast_to([P, EC, 64])
                            sb_ = sin_all[:, 64 * sr:64 * (sr + 1)].rearrange(
                                "p (o j) -> p o j", o=1).broadcast_to([P, EC, 64])
                            nc.vector.scalar_tensor_tensor(
                                out=tr[:, :, 0], in0=xr[:, :, 1], scalar=s_t[:],
                                in1=sb_, op0=OP.mult, op1=OP.mult)
                            nc.vector.scalar_tensor_tensor(
                                out=yr[:, :, 0], in0=xr[:, :, 0], scalar=s_t[:],
                                in1=cb, op0=OP.mult, op1=OP.mult)
                            nc.vector.tensor_tensor(
                                out=yr[:, :, 0], in0=yr[:, :, 0], in1=tr[:, :, 0],
                                op=OP.subtract)
                            nc.vector.scalar_tensor_tensor(
                                out=tr[:, :, 1], in0=xr[:, :, 0], scalar=s_t[:],
                                in1=sb_, op0=OP.mult, op1=OP.mult)
                            nc.vector.scalar_tensor_tensor(
                                out=yr[:, :, 1], in0=xr[:, :, 1], scalar=s_t[:],
                                in1=cb, op0=OP.mult, op1=OP.mult)
                            nc.vector.tensor_tensor(
                                out=yr[:, :, 1], in0=yr[:, :, 1], in1=tr[:, :, 1],
                                op=OP.add)
                            # transpose 16 chunks -> yT (4 psum groups of 4)
                            for gch in range(4):
                                tps = st_ps.tile([P, 512], BF, tag="tps")
                                for c4 in range(4):
                                    c = 4 * gch + c4
                                    nc.tensor.transpose(
                                        tps[:, P * c4:P * (c4 + 1)],
                                        y_t[:, P * c:P * (c + 1)], ident[:])
                                yT_dst = yT[:].rearrange("p (c t) -> p c t", t=ATB)
                                nc.scalar.activation(
                                    yT_dst[:, 4 * gch:4 * gch + 4, P * tt:P * (tt + 1)],
                                    tps[:].rearrange("p (c t) -> p c t", t=P),
                                    AF.Copy, scale=16.0)
                        # QKV matmuls (6 groups of 128 cols)
                        for half in range(2):
                            for g3 in range(3):
                                g = 3 * half + g3
                                pq = qkv_ps.tile([P, ATB], F32, tag=f"pq{g3 % 2}")
                                wv_v = wqkv_sb[:].rearrange(
                                    "p (e c) -> p e c", e=EC)
                                yT_v = yT[:].rearrange("p (e t) -> p e t", e=EC)
                                for ep in range(EC // 2):
                                    nc.tensor.matmul(
                                        pq[:],
                                        wv_v[:, 2 * ep:2 * ep + 2, P * g:P * (g + 1)],
                                        yT_v[:, 2 * ep:2 * ep + 2, :],
                                        start=(ep == 0), stop=(ep == EC // 2 - 1),
                                        perf_mode=DR)
                                h = g % 2
                                col = S * h + ATB * tb
                                if g < 2:      # Q heads (scale folded host-side)
                                    nc.scalar.activation(qt_b[:, col:col + ATB],
                                                         pq[:], AF.Copy,
                                                         scale=1.0 / 8192.0)
                                elif g < 4:    # K heads
                                    nc.scalar.activation(kt_b[:, col:col + ATB],
                                                         pq[:], AF.Copy,
                                                         scale=1.0 / 8192.0)
                                else:          # V heads -> transpose to [tok, DH]
                                    vt_tmp = st_sb.tile([P, ATB], BF, tag="vt")
                                    nc.scalar.activation(vt_tmp[:], pq[:], AF.Copy,
                                                         scale=1.0 / 512.0)
                                    tpv = st_ps.tile([P, 512], BF, tag="tps")
                                    for j in range(ATB // P):
                                        nc.tensor.transpose(
                                            tpv[:, P * j:P * (j + 1)],
                                            vt_tmp[:, P * j:P * (j + 1)], ident[:])
                                    nc.vector.tensor_copy(v_b[:, col:col + ATB],
                                                          tpv[:, 0:ATB])
                    # ---- attention for batch b, both local heads ----
                    for h in range(HLOC):
                        hs = S * h
                        for qb in range(4):
                            nk = 4 * qb + 4
                            ps_o = at_o_ps.tile([P, 512], F32, tag="pso")
                            ps_d = at_db_ps.tile([1, 512], F32, tag="db")
                            nkp = nk // 2
                            for ktp in range(nkp):
                                probs = at_sb.tile([P, 1024], F8, tag="probs")
                                for sub in range(2):
                                    kt = 2 * ktp + sub
                                    ps_s = at_s_ps.tile([P, 512], F32, tag="pss")
                                    nc.tensor.matmul(
                                        ps_s[:],
                                        kt_b[:, hs + P * kt: hs + P * (kt + 1)],
                                        qt_b[:, hs + 512 * qb: hs + 512 * (qb + 1)],
                                        start=True, stop=True)
                                    pr = probs[:, 512 * sub:512 * (sub + 1)]
                                    nc.scalar.activation(pr, ps_s[:], AF.Exp,
                                                         bias=neg1[:])
                                    if kt >= 4 * qb:
                                        m = kt - 4 * qb
                                        nc.vector.tensor_tensor(
                                            out=pr, in0=pr,
                                            in1=masks[:, 512 * m:512 * (m + 1)],
                                            op=OP.mult)
                                pr2 = probs[:].rearrange(
                                    "p (two t) -> p two t", two=2)
                                nc.tensor.matmul(
                                    ps_o[:],
                                    v_b[:, hs + 2 * P * ktp: hs + 2 * P * (ktp + 1)].rearrange(
                                        "p (two t) -> p two t", two=2),
                                    pr2,
                                    start=(ktp == 0), stop=(ktp == nkp - 1),
                                    skip_group_check=True, perf_mode=DR)
                                nc.tensor.matmul(
                                    ps_d[:],
                                    ones_col[:].rearrange(
                                        "p (two t) -> p two t", two=2)[:, :, 0:1],
                                    pr2,
                                    start=(ktp == 0), stop=(ktp == nkp - 1),
                                    skip_group_check=True, perf_mode=DR)
                            rd = scr_p.tile([1, 512], F32R, tag="rd")
                            with nc.allow_low_precision(reason="softmax denom"):
                                nc.vector.reciprocal(rd[:], ps_d[:])
                            ps_b = at_db_ps.tile([P, 512], F32, tag="db")
                            nc.tensor.matmul(ps_b[:], ones_row[:], rd[:],
                                             start=True, stop=True)
                            osb = at_sb.tile([P, 512], BF, tag="osb")
                            nc.scalar.copy(osb[:], ps_o[:])
                            ot = at_sb.tile([P, 512], BF, tag="ot")
                            nc.vector.tensor_tensor(out=ot[:], in0=osb[:],
                                                    in1=ps_b[:], op=OP.mult)
                            # tokens 512*qb..512*qb+512 of batch b ->
                            # dest cores 2qb (first 256) and 2qb+1
                            a2a_v = a2a_in[b][:].rearrange(
                                "(d r) t -> d r t", d=NCORES)
                            for m in range(2):
                                nc.sync.dma_start(
                                    out=a2a_v[2 * qb + m, P * h:P * (h + 1), :],
                                    in_=ot[:, TB * m:TB * (m + 1)])
                    nc.gpsimd.collective_compute(
                        "AllToAll", OP.bypass, replica_groups=RG,
                        ins=[a2a_in[b][:]], outs=[a2a_out[b][:]])
                stgA.close()

                # ================= phase B: WO + residual + FFN norm ========
                # fp32 accumulator tiles hold x + attn@wo, later += FFN out.
                acc = []
                for t8 in range(8):
                    a = acc_p.tile([P, E], F32, name=f"acc{t8}")
                    nc.gpsimd.dma_start(out=a[:], in_=xsl_in[P * t8:P * (t8 + 1), :])
                    acc.append(a)
                y2T = y2T_p.tile([P, EC * TSL], BF)

                phB = ExitStack()
                ot_p = phB.enter_context(tc.tile_pool(name="ot_p", bufs=1))
                wo_p = phB.enter_context(tc.tile_pool(name="wo_p", bufs=1))
                wo_sb = phB.enter_context(tc.tile_pool(name="wo_sb", bufs=2))
                wo_ps = phB.enter_context(tc.tile_pool(name="wo_ps", bufs=2, space="PSUM"))
                wo_tps = phB.enter_context(tc.tile_pool(name="wo_tps", bufs=2, space="PSUM"))

                ot_sb = []
                for b in range(B):
                    o = ot_p.tile([P, H * TB], BF, name=f"otb{b}")
                    nc.gpsimd.dma_start(
                        out=o[:].rearrange("p (hc t) -> p hc t", hc=H),
                        in_=a2a_out[b][:].rearrange("(hc p) t -> p hc t", p=P))
                    o8 = ot_p.tile([P, H * TB], F8, name=f"otb8{b}")
                    nc.scalar.copy(o8[:], o[:])
                    ot_sb.append(o8)

                wo_full = wo_p.tile([P, H * E], F8, tag="wof")
                nc.gpsimd.dma_start(
                    out=wo_full[:].rearrange("p (hc w) -> p hc w", hc=H),
                    in_=wo_in[:].rearrange("hc p w -> p hc w"))
                wo_v = wo_full[:].rearrange("p (hc w) -> p hc w", hc=H)
                for t8 in range(8):
                    b, tt = t8 // 2, t8 % 2
                    ot_v = ot_sb[b][:].rearrange("p (hc t) -> p hc t", hc=H)
                    for ecol in range(4):
                        ps = wo_ps.tile([P, 512], F32, tag="ps")
                        for hp in range(H // 2):
                            nc.tensor.matmul(
                                ps[:],
                                ot_v[:, 2 * hp:2 * hp + 2, P * tt:P * (tt + 1)],
                                wo_v[:, 2 * hp:2 * hp + 2,
                                     512 * ecol:512 * (ecol + 1)],
                                start=(hp == 0), stop=(hp == H // 2 - 1),
                                perf_mode=DR)
                        nc.vector.scalar_tensor_tensor(
                            out=acc[t8][:, 512 * ecol:512 * (ecol + 1)],
                            in0=ps[:], scalar=c_wo[:],
                            in1=acc[t8][:, 512 * ecol:512 * (ecol + 1)],
                            op0=OP.mult, op1=OP.add)
                # FFN rmsnorm + transpose -> y2T [e, tok]
                for t8 in range(8):
                    scr2 = wo_sb.tile([P, E], BF, tag="scr2")
                    ssq2 = wo_sb.tile([P, 1], F32, tag="ssq2")
                    nc.scalar.activation(scr2[:], acc[t8][:], AF.Square,
                                         accum_out=ssq2[:])
                    sq2 = wo_sb.tile([P, 1], F32, tag="sq2")
                    nc.scalar.activation(sq2[:], ssq2[:], AF.Sqrt,
                                         scale=1.0 / E, bias=eps_t[:])
                    s2 = wo_sb.tile([P, 1], F32, tag="s2")
                    nc.vector.reciprocal(s2[:], sq2[:])
                    y2_t = wo_sb.tile([P, E], BF, tag="y2")
                    nc.scalar.activation(y2_t[:], acc[t8][:], AF.Copy,
                                         scale=s2[:])
                    y2T_v = y2T[:].rearrange("p (c t) -> p c t", t=TSL)
                    for gch in range(4):
                        tps = wo_tps.tile([P, 512], BF, tag="tps")
                        for c4 in range(4):
                            c = 4 * gch + c4
                            nc.tensor.transpose(
                                tps[:, P * c4:P * (c4 + 1)],
                                y2_t[:, P * c:P * (c + 1)], ident[:])
                        nc.vector.tensor_copy(
                            y2T_v[:, 4 * gch:4 * gch + 4, P * t8:P * (t8 + 1)],
                            tps[:].rearrange("p (c t) -> p c t", t=P))
                phB.close()

                # ================= phase C: FFN, streamed full weights ======
                phC = ExitStack()
                wf_p = phC.enter_context(tc.tile_pool(name="wf_p", bufs=2))
                hT_p = phC.enter_context(tc.tile_pool(name="hT_p", bufs=1))
                f1_sb = phC.enter_context(tc.tile_pool(name="f1_sb", bufs=2))
                f1_gps = phC.enter_context(tc.tile_pool(name="f1_gps", bufs=2, space="PSUM"))
                f1_lps = phC.enter_context(tc.tile_pool(name="f1_lps", bufs=2, space="PSUM"))
                f2_ps = phC.enter_context(tc.tile_pool(name="f2_ps", bufs=2, space="PSUM"))

                for k in range(FBLK):
                    hT = hT_p.tile([P, FCB * TSL], BF, tag="hT")
                    for fc in range(FCB):
                        wgs = wf_p.tile([P, EC * P], BF, tag="wg")
                        nc.sync.dma_start(
                            out=wgs[:].rearrange("p (e c) -> p e c", e=EC),
                            in_=wg_in[FCB * k + fc].rearrange("e p c -> p e c"))
                        wls = wf_p.tile([P, EC * P], BF, tag="wl")
                        nc.sync.dma_start(
                            out=wls[:].rearrange("p (e c) -> p e c", e=EC),
                            in_=wl_in[FCB * k + fc].rearrange("e p c -> p e c"))
                        for g2 in range(2):   # 512-token groups
                            psg = f1_gps.tile([P, 512], F32, tag="psg")
                            psl = f1_lps.tile([P, 512], F32, tag="psl")
                            for ec in range(EC):
                                mv = y2T[:, TSL * ec + 512 * g2:
                                         TSL * ec + 512 * (g2 + 1)]
                                nc.tensor.matmul(
                                    psg[:], wgs[:, P * ec:P * (ec + 1)], mv,
                                    start=(ec == 0), stop=(ec == EC - 1),
                                    skip_group_check=True)
                                nc.tensor.matmul(
                                    psl[:], wls[:, P * ec:P * (ec + 1)], mv,
                                    start=(ec == 0), stop=(ec == EC - 1),
                                    skip_group_check=True)
                            tmp_g = f1_sb.tile([P, 512], BF, tag="tmpg")
                            nc.scalar.activation(tmp_g[:], psg[:], AF.Gelu)
                            nc.vector.tensor_tensor(
                                out=hT[:, TSL * fc + 512 * g2:
                                       TSL * fc + 512 * (g2 + 1)],
                                in0=tmp_g[:], in1=psl[:], op=OP.mult)
                    # F2: out partial [tok, E] accumulated into acc
                    for ecol in range(4):
                        wos = wf_p.tile([P, FCB * 512], BF, tag="wos")
                        nc.sync.dma_start(
                            out=wos[:].rearrange("p (f w) -> p f w", f=FCB),
                            in_=wout_in[FCB * k:FCB * (k + 1), :,
                                        512 * ecol:512 * (ecol + 1)].rearrange(
                                "f p w -> p f w"))
                        for t8 in range(8):
                            ps2 = f2_ps.tile([P, 512], F32, tag="ps2")
                            for fs in range(FCB):
                                nc.tensor.matmul(
                                    ps2[:],
                                    hT[:, TSL * fs + P * t8: TSL * fs + P * (t8 + 1)],
                                    wos[:, 512 * fs:512 * (fs + 1)],
                                    start=(fs == 0), stop=(fs == FCB - 1))
                            nc.vector.tensor_tensor(
                                out=acc[t8][:, 512 * ecol:512 * (ecol + 1)],
                                in0=ps2[:],
                                in1=acc[t8][:, 512 * ecol:512 * (ecol + 1)],
                                op=OP.add)
                            if k == FBLK - 1 and ecol == 3:
                                nc.sync.dma_start(
                                    out=out_sl[P * t8:P * (t8 + 1), :],
                                    in_=acc[t8][:])
                phC.close()
    nc.compile()
    return nc, names


def _prep_inputs_shared(inputs):
    """Host-side prep of tensors identical on every core."""
    import ml_dtypes
    BF = ml_dtypes.bfloat16
    x = np.ascontiguousarray(
        np.asarray(inputs["inputs"], np.float32).reshape(TOK, E))
    wo = np.asarray(inputs["wo"], np.float32)
    w_gate = np.asarray(inputs["w_gate"], np.float32)
    w_lin = np.asarray(inputs["w_lin"], np.float32)
    w_out = np.asarray(inputs["w_out"], np.float32)
    gamma_attn = np.asarray(inputs["gamma_attn"], np.float32)
    gamma_ffn = np.asarray(inputs["gamma_ffn"], np.float32)
    positions = np.asarray(inputs["positions"])

    F8 = ml_dtypes.float8_e4m3
    x_bf = np.ascontiguousarray(x.astype(BF))
    wo_r = np.ascontiguousarray((wo.reshape(H, P, E) * 256.0).astype(F8))
    # [E, F] -> [FC, EC, P, P] (fcol-major strips)
    def _gl(w):
        w = (w * gamma_ffn[:, None]).astype(BF)
        return np.ascontiguousarray(
            w.reshape(EC, P, FC, P).transpose(2, 0, 1, 3))
    wg_t = _gl(w_gate)
    wl_t = _gl(w_lin)
    wout_t = np.ascontiguousarray(w_out.reshape(FC, P, E).astype(BF))

    assert np.all(gamma_attn == gamma_attn[0]), \
        "non-uniform gamma_attn needs full-width rope tables"
    half = DH // 2
    inv_freq = (1.0 / (10000.0 ** (np.arange(half, dtype=np.float32) / half))
                ).astype(np.float32)
    ang = positions.astype(np.float32)[:, None] * inv_freq[None, :]
    g0 = float(gamma_attn[0])
    cos = (np.cos(ang) * g0).astype(BF)
    sin = (np.sin(ang) * g0).astype(BF)

    k_i = np.arange(P)[:, None]
    q_i = np.arange(512)[None, :]
    msk = np.stack([(P * m + k_i <= q_i) for m in range(4)]).astype(F8)

    return {
        "x": x_bf, "wo": wo_r, "wg": wg_t, "wl": wl_t, "wout": wout_t,
        "cos": cos, "sin": sin, "mask": msk,
        "onec": np.ones((P, 32), F8),
        "oner": np.ones((1, P), np.float32),
    }, x


def _prep_inputs_core(inputs, x_f32, r):
    """Per-core tensors: head-sharded QKV weights + owned-token x slice."""
    import ml_dtypes
    F8 = ml_dtypes.float8_e4m3
    wq = np.asarray(inputs["wq"], np.float32) / np.sqrt(np.float32(DH))
    wk = np.asarray(inputs["wk"], np.float32)
    wv = np.asarray(inputs["wv"], np.float32)
    h0 = HLOC * r

    def _slice_qkv(w):   # [E, H, DH] -> [EC, P, HLOC*DH]
        return w[:, h0:h0 + HLOC, :].reshape(EC, P, HLOC * DH)

    wqkv = (np.concatenate([_slice_qkv(wq), _slice_qkv(wk), _slice_qkv(wv)],
                           axis=2) * 512.0).astype(F8)
    # tokens owned by core r: 256 from each batch
    xsl = np.ascontiguousarray(
        x_f32.reshape(B, NCORES, TB, E)[:, r].reshape(TSL, E))
    return {"wqkv": np.ascontiguousarray(wqkv), "xsl": xsl}


def _run(inputs, trace=False):
    from concourse.bass_utils import run_bass_kernel_spmd

    if "nc" not in _CACHE:
        _CACHE["nc"], _CACHE["names"] = _build()
    nc, names = _CACHE["nc"], _CACHE["names"]

    shared, x_f32 = _prep_inputs_shared(inputs)
    in_maps = []
    for r in range(NCORES):
        prep = dict(shared)
        prep.update(_prep_inputs_core(inputs, x_f32, r))
        in_maps.append({names[k]: v for k, v in prep.items()})

    res = run_bass_kernel_spmd(nc, in_maps, core_ids=list(range(NCORES)),
                               trace=trace)
    out = np.empty((B, NCORES, TB, E), np.float32)
    for r in range(NCORES):
        out[:, r] = res.results[r][names["out"]].reshape(B, TB, E)
    return out.reshape(B, S, E), res


def kernel(**inputs) -> np.ndarray:
    return _run(inputs)[0]


# revision 28
# speedup vs baseline: 1.1400x; 1.0643x over previous
"""Trainium2 Bass kernel for a dense transformer decoder block.

Tensor-parallel over 8 NeuronCores, bf16 matmuls (fp32 accumulation):
  Phase A: heads sharded (2/core). norm+rope+QKV+causal attention over all
           tokens; per-batch AllToAll redistributes attention output to
           token owners (each core owns 256 tokens of each batch).
  Phase B: WO projection + residual into a persistent fp32 accumulator,
           then FFN rmsnorm -> y2^T.
  Phase C: FFN over the full hidden dim with streamed bf16 weights
           (no collectives); F2 accumulates into the fp32 accumulator.
"""
import sys

if '/opt/trn_rl_repo' not in sys.path:
    sys.path.insert(0, '/opt/trn_rl_repo')

import numpy as np
from contextlib import ExitStack

B, S, E, H, DH, F = 4, 2048, 2048, 16, 128, 8192
P = 128
NCORES = 8
HLOC = H // NCORES          # 2 heads per core
TOK = B * S                 # 8192 tokens
TSL = TOK // NCORES         # 1024 tokens per core (256 from each batch)
TB = TSL // B               # 256 tokens per (core, batch)
EC = E // P                 # 16 embedding chunks
FC = F // P                 # 64 FFN col chunks
FBLK = 4                    # outer F blocks
FCB = FC // FBLK            # 16 col chunks per block
EPS = 1e-5
ATB = 256                   # phase-A token block
NTB = S // ATB              # 8 blocks per batch

_CACHE = {}


def _build():
    import concourse.bacc as bacc
    import concourse.mybir as mybir
    import concourse.tile as tile
    import concourse.tile_utils as tile_utils
    from concourse.masks import make_identity

    tile_utils.max_sbuf_usage = 204 * 1024

    F32 = mybir.dt.float32
    F32R = mybir.dt.float32r
    BF = mybir.dt.bfloat16
    F8 = mybir.dt.float8e4
    DR = mybir.MatmulPerfMode.DoubleRow
    AF = mybir.ActivationFunctionType
    OP = mybir.AluOpType

    nc = bacc.Bacc(None, target_bir_lowering=False)
    names = {}

    with tile.TileContext(nc) as tc:
        with tc.tile_pool(name="dram", bufs=1, space="DRAM") as dram:
            # ---- external inputs ----
            xbf_in = dram.tile([TSL, E], BF, kind="ExternalInput")
            xsl_in = dram.tile([TSL, E], F32, kind="ExternalInput")
            wqkv_in = dram.tile([EC, P, 6 * P], F8, kind="ExternalInput")
            wo_in = dram.tile([H, P, E], F8, kind="ExternalInput")
            wg_in = dram.tile([FC, EC, P, P], BF, kind="ExternalInput")
            wl_in = dram.tile([FC, EC, P, P], BF, kind="ExternalInput")
            wout_in = dram.tile([FC, P, E], BF, kind="ExternalInput")
            cos_in = dram.tile([TB, 64], BF, kind="ExternalInput")
            sin_in = dram.tile([TB, 64], BF, kind="ExternalInput")
            mask_in = dram.tile([4, P, 512], F8, kind="ExternalInput")
            onec_in = dram.tile([P, 32], F8, kind="ExternalInput")
            oner_in = dram.tile([1, P], F32R, kind="ExternalInput")
            out_sl = dram.tile([TSL, E], F32, kind="ExternalOutput")
            names.update(
                x=xbf_in.name, xsl=xsl_in.name, wqkv=wqkv_in.name, wo=wo_in.name,
                wg=wg_in.name, wl=wl_in.name, wout=wout_in.name,
                cos=cos_in.name, sin=sin_in.name, mask=mask_in.name,
                onec=onec_in.name, oner=oner_in.name, out=out_sl.name)

            # ---- internal DRAM: per-batch AllToAll bounce ----
            a2a_in = [dram.tile([NCORES * HLOC * P, TB], BF, name=f"a2ai{b}")
                      for b in range(B)]
            agy_in = [dram.tile([E, TB], F8, name=f"agyi{b}")
                      for b in range(B)]
            agy_out = [dram.tile([NCORES * E, TB], F8, name=f"agyo{b}",
                                 addr_space="Shared")
                       for b in range(B)]
            a2a_out = [dram.tile([NCORES * HLOC * P, TB], BF,
                                 name=f"a2ao{b}")
                       for b in range(B)]

            RG = [list(range(NCORES))]

            with tc.tile_pool(name="cst", bufs=1) as cst, \
                 tc.tile_pool(name="acc_p", bufs=1) as acc_p, \
                 tc.tile_pool(name="y2T_p", bufs=1) as y2T_p:
                ident = cst.tile([P, P], BF)
                make_identity(nc, ident[:])
                eps_t = cst.tile([P, 1], F32)
                nc.gpsimd.memset(eps_t[:], EPS)
                neg1 = cst.tile([P, 1], F32)
                nc.gpsimd.memset(neg1[:], -1.0)
                c_wo = cst.tile([P, 1], F32)
                nc.gpsimd.memset(c_wo[:], 1.0 / 4096.0)

                # ================= phase A: norm+rope+QKV+attention =========
                stgA = ExitStack()
                wqkv_p = stgA.enter_context(tc.tile_pool(name="wqkv_p", bufs=1))
                tabs = stgA.enter_context(tc.tile_pool(name="tabs", bufs=1))
                ytb_p = stgA.enter_context(tc.tile_pool(name="ytb", bufs=2))
                qkvb_p = stgA.enter_context(tc.tile_pool(name="qkvb", bufs=1))
                st_sb = stgA.enter_context(tc.tile_pool(name="st_sb", bufs=2))
                scr_p = stgA.enter_context(tc.tile_pool(name="scr_p", bufs=1))
                st_ps = stgA.enter_context(tc.tile_pool(name="st_ps", bufs=2, space="PSUM"))
                qkv_ps = stgA.enter_context(tc.tile_pool(name="qkv_ps", bufs=1, space="PSUM"))
                at_s_ps = stgA.enter_context(tc.tile_pool(name="at_s_ps", bufs=2, space="PSUM"))
                at_o_ps = stgA.enter_context(tc.tile_pool(name="at_o_ps", bufs=1, space="PSUM"))
                at_db_ps = stgA.enter_context(tc.tile_pool(name="at_db_ps", bufs=1, space="PSUM"))
                at_sb = stgA.enter_context(tc.tile_pool(name="at_sb", bufs=2))

                wqkv_sb = wqkv_p.tile([P, EC * 6 * P], F8)
                nc.sync.dma_start(
                    out=wqkv_sb[:].rearrange("p (e c) -> p e c", e=EC),
                    in_=wqkv_in[:].rearrange("e p c -> p e c"))
                ones_col = tabs.tile([P, 32], F8)
                nc.sync.dma_start(out=ones_col[:], in_=onec_in[:])
                ones_row = tabs.tile([1, P], F32R)
                nc.sync.dma_start(out=ones_row[:], in_=oner_in[:])
                masks = tabs.tile([P, 4 * 512], F8)
                nc.sync.dma_start(
                    out=masks[:].rearrange("p (m w) -> p m w", m=4),
                    in_=mask_in[:].rearrange("m p w -> p m w"))
                # rope tables for own 256 positions -> sbuf [pos%128, (blk, j)]
                cos_all = tabs.tile([P, 2 * 64], BF)
                sin_all = tabs.tile([P, 2 * 64], BF)
                nc.sync.dma_start(
                    out=cos_all[:].rearrange("p (r j) -> p r j", r=2),
                    in_=cos_in[:].rearrange("(r p) j -> p r j", p=P))
                nc.sync.dma_start(
                    out=sin_all[:].rearrange("p (r j) -> p r j", r=2),
                    in_=sin_in[:].rearrange("(r p) j -> p r j", p=P))

                def prep_batch(pb):
                    """norm+rope+transpose own 256 tokens of batch pb,
                    write yT slice to agy_in[pb], then AllGather."""
                    for tt in range(2):
                        row = TB * pb + P * tt
                        x_t = st_sb.tile([P, E], BF, tag="x")
                        nc.sync.dma_start(out=x_t[:], in_=xbf_in[row:row + P, :])
                        scr = scr_p.tile([P, E], BF, tag="scr")
                        ssq = st_sb.tile([P, 1], F32, tag="ssq")
                        nc.scalar.activation(scr[:], x_t[:], AF.Square,
                                             accum_out=ssq[:])
                        sq = st_sb.tile([P, 1], F32, tag="sq")
                        nc.scalar.activation(sq[:], ssq[:], AF.Sqrt,
                                             scale=1.0 / E, bias=eps_t[:])
                        s_t = st_sb.tile([P, 1], F32, tag="s")
                        nc.vector.reciprocal(s_t[:], sq[:])
                        y_t = st_sb.tile([P, E], BF, tag="y")
                        t1 = st_sb.tile([P, E], BF, tag="t1")
                        xr = x_t[:].rearrange("p (c two h) -> p c two h", two=2, h=64)
                        yr = y_t[:].rearrange("p (c two h) -> p c two h", two=2, h=64)
                        tr = t1[:].rearrange("p (c two h) -> p c two h", two=2, h=64)
                        cb = cos_all[:, 64 * tt:64 * (tt + 1)].rearrange(
                            "p (o j) -> p o j", o=1).broadcast_to([P, EC, 64])
                        sb_ = sin_all[:, 64 * tt:64 * (tt + 1)].rearrange(
                            "p (o j) -> p o j", o=1).broadcast_to([P, EC, 64])
                        nc.vector.scalar_tensor_tensor(
                            out=tr[:, :, 0], in0=xr[:, :, 1], scalar=s_t[:],
                            in1=sb_, op0=OP.mult, op1=OP.mult)
                        nc.vector.scalar_tensor_tensor(
                            out=yr[:, :, 0], in0=xr[:, :, 0], scalar=s_t[:],
                            in1=cb, op0=OP.mult, op1=OP.mult)
                        nc.vector.tensor_tensor(
                            out=yr[:, :, 0], in0=yr[:, :, 0], in1=tr[:, :, 0],
                            op=OP.subtract)
                        nc.vector.scalar_tensor_tensor(
                            out=tr[:, :, 1], in0=xr[:, :, 0], scalar=s_t[:],
                            in1=sb_, op0=OP.mult, op1=OP.mult)
                        nc.vector.scalar_tensor_tensor(
                            out=yr[:, :, 1], in0=xr[:, :, 1], scalar=s_t[:],
                            in1=cb, op0=OP.mult, op1=OP.mult)
                        nc.vector.tensor_tensor(
                            out=yr[:, :, 1], in0=yr[:, :, 1], in1=tr[:, :, 1],
                            op=OP.add)
                        yTo = st_sb.tile([P, EC * P], F8, tag="yTo")
                        yTo_v = yTo[:].rearrange("p (c t) -> p c t", t=P)
                        for gch in range(4):
                            tps = st_ps.tile([P, 512], BF, tag="tps")
                            for c4 in range(4):
                                c = 4 * gch + c4
                                nc.tensor.transpose(
                                    tps[:, P * c4:P * (c4 + 1)],
                                    y_t[:, P * c:P * (c + 1)], ident[:])
                            nc.scalar.activation(
                                yTo_v[:, 4 * gch:4 * gch + 4, :],
                                tps[:].rearrange("p (c t) -> p c t", t=P),
                                AF.Copy, scale=16.0)
                        nc.sync.dma_start(
                            out=agy_in[pb][:].rearrange(
                                "(ec p) t -> p ec t", p=P)[:, :, P * tt:P * (tt + 1)],
                            in_=yTo_v)
                    nc.gpsimd.collective_compute(
                        "AllGather", OP.bypass, replica_groups=RG,
                        ins=[agy_in[pb][:]], outs=[agy_out[pb][:]])

                prep_batch(0)
                for b in range(B):
                    qt_b = qkvb_p.tile([P, HLOC * S], BF, tag="qt")
                    kt_b = qkvb_p.tile([P, HLOC * S], BF, tag="kt")
                    v_b = qkvb_p.tile([P, HLOC * S], F8, tag="vb")
                    agy_v = agy_out[b][:].rearrange(
                        "(s ec p) t -> s ec p t", s=NCORES, p=P)
                    for tb in range(NTB):    # 256-token blocks (= src core tb)
                        yT = ytb_p.tile([P, EC * ATB], F8, tag="yT")
                        nc.sync.dma_start(
                            out=yT[:].rearrange("p (e t) -> p e t", e=EC),
                            in_=agy_v[tb].rearrange("ec p t -> p ec t"))
                        # QKV matmuls (6 groups of 128 cols)
                        for half in range(2):
                            for g3 in range(3):
                                g = 3 * half + g3
                                pq = qkv_ps.tile([P, ATB], F32, tag=f"pq{g3 % 2}")
                                wv_v = wqkv_sb[:].rearrange(
                                    "p (e c) -> p e c", e=EC)
                                yT_v = yT[:].rearrange("p (e t) -> p e t", e=EC)
                                for ep in range(EC // 2):
                                    nc.tensor.matmul(
                                        pq[:],
                                        wv_v[:, 2 * ep:2 * ep + 2, P * g:P * (g + 1)],
                                        yT_v[:, 2 * ep:2 * ep + 2, :],
                                        start=(ep == 0), stop=(ep == EC // 2 - 1),
                                        perf_mode=DR)
                                h = g % 2
                                col = S * h + ATB * tb
                                if g < 2:      # Q heads (scale folded host-side)
                                    nc.scalar.activation(qt_b[:, col:col + ATB],
                                                         pq[:], AF.Copy,
                                                         scale=1.0 / 8192.0)
                                elif g < 4:    # K heads
                                    nc.scalar.activation(kt_b[:, col:col + ATB],
                                                         pq[:], AF.Copy,
                                                         scale=1.0 / 8192.0)
                                else:          # V heads -> transpose to [tok, DH]
                                    vt_tmp = st_sb.tile([P, ATB], BF, tag="vt")
                                    nc.scalar.activation(vt_tmp[:], pq[:], AF.Copy,
                                                         scale=1.0 / 512.0)
                                    tpv = st_ps.tile([P, 512], BF, tag="tps")
                                    for j in range(ATB // P):
                                        nc.tensor.transpose(
                                            tpv[:, P * j:P * (j + 1)],
                                            vt_tmp[:, P * j:P * (j + 1)], ident[:])
                                    nc.vector.tensor_copy(v_b[:, col:col + ATB],
                                                          tpv[:, 0:ATB])
                    if b + 1 < B:
                        prep_batch(b + 1)
                    # ---- attention for batch b, both local heads ----
                    for h in range(HLOC):
                        hs = S * h
                        for qb in range(4):
                            nk = 4 * qb + 4
                            ps_o = at_o_ps.tile([P, 512], F32, tag="pso")
                            ps_d = at_db_ps.tile([1, 512], F32, tag="db")
                            nkp = nk // 2
                            for ktp in range(nkp):
                                probs = at_sb.tile([P, 1024], F8, tag="probs")
                                for sub in range(2):
                                    kt = 2 * ktp + sub
                                    ps_s = at_s_ps.tile([P, 512], F32, tag="pss")
                                    nc.tensor.matmul(
                                        ps_s[:],
                                        kt_b[:, hs + P * kt: hs + P * (kt + 1)],
                                        qt_b[:, hs + 512 * qb: hs + 512 * (qb + 1)],
                                        start=True, stop=True)
                                    pr = probs[:, 512 * sub:512 * (sub + 1)]
                                    nc.scalar.activation(pr, ps_s[:], AF.Exp,
                                                         bias=neg1[:])
                                    if kt >= 4 * qb:
                                        m = kt - 4 * qb
                                        nc.vector.tensor_tensor(
                                            out=pr, in0=pr,
                                            in1=masks[:, 512 * m:512 * (m + 1)],
                                            op=OP.mult)
                                pr2 = probs[:].rearrange(
                                    "p (two t) -> p two t", two=2)
                                nc.tensor.matmul(
                                    ps_o[:],
                                    v_b[:, hs + 2 * P * ktp: hs + 2 * P * (ktp + 1)].rearrange(
                                        "p (two t) -> p two t", two=2),
                                    pr2,
                                    start=(ktp == 0), stop=(ktp == nkp - 1),
                                    skip_group_check=True, perf_mode=DR)
                                nc.tensor.matmul(
                                    ps_d[:],
                                    ones_col[:].rearrange(
                                        "p (two t) -> p two t", two=2)[:, :, 0:1],
                                    pr2,
                                    start=(ktp == 0), stop=(ktp == nkp - 1),
                                    skip_group_check=True, perf_mode=DR)
                            rd = scr_p.tile([1, 512], F32R, tag="rd")
                            with nc.allow_low_precision(reason="softmax denom"):
                                nc.vector.reciprocal(rd[:], ps_d[:])
                            ps_b = at_db_ps.tile([P, 512], F32, tag="db")
                            nc.tensor.matmul(ps_b[:], ones_row[:], rd[:],
                                             start=True, stop=True)
                            osb = at_sb.tile([P, 512], BF, tag="osb")
                            nc.scalar.copy(osb[:], ps_o[:])
                            ot = at_sb.tile([P, 512], BF, tag="ot")
                            nc.vector.tensor_tensor(out=ot[:], in0=osb[:],
                                                    in1=ps_b[:], op=OP.mult)
                            # tokens 512*qb..512*qb+512 of batch b ->
                            # dest cores 2qb (first 256) and 2qb+1
                            a2a_v = a2a_in[b][:].rearrange(
                                "(d r) t -> d r t", d=NCORES)
                            for m in range(2):
                                nc.sync.dma_start(
                                    out=a2a_v[2 * qb + m, P * h:P * (h + 1), :],
                                    in_=ot[:, TB * m:TB * (m + 1)])
                    nc.gpsimd.collective_compute(
                        "AllToAll", OP.bypass, replica_groups=RG,
                        ins=[a2a_in[b][:]], outs=[a2a_out[b][:]])
                stgA.close()

                # ================= phase B: WO + residual + FFN norm ========
                # fp32 accumulator tiles hold x + attn@wo, later += FFN out.
                acc = []
                for t8 in range(8):
                    a = acc_p.tile([P, E], F32, name=f"acc{t8}")
                    nc.gpsimd.dma_start(out=a[:], in_=xsl_in[P * t8:P * (t8 + 1), :])
                    acc.append(a)
                y2T = y2T_p.tile([P, EC * TSL], BF)

                phB = ExitStack()
                ot_p = phB.enter_context(tc.tile_pool(name="ot_p", bufs=1))
                wo_p = phB.enter_context(tc.tile_pool(name="wo_p", bufs=1))
                wo_sb = phB.enter_context(tc.tile_pool(name="wo_sb", bufs=2))
                wo_ps = phB.enter_context(tc.tile_pool(name="wo_ps", bufs=2, space="PSUM"))
                wo_tps = phB.enter_context(tc.tile_pool(name="wo_tps", bufs=2, space="PSUM"))

                ot_sb = []
                for b in range(B):
                    o = ot_p.tile([P, H * TB], BF, name=f"otb{b}")
                    nc.gpsimd.dma_start(
                        out=o[:].rearrange("p (hc t) -> p hc t", hc=H),
                        in_=a2a_out[b][:].rearrange("(hc p) t -> p hc t", p=P))
                    o8 = ot_p.tile([P, H * TB], F8, name=f"otb8{b}")
                    nc.scalar.copy(o8[:], o[:])
                    ot_sb.append(o8)

                wo_full = wo_p.tile([P, H * E], F8, tag="wof")
                nc.gpsimd.dma_start(
                    out=wo_full[:].rearrange("p (hc w) -> p hc w", hc=H),
                    in_=wo_in[:].rearrange("hc p w -> p hc w"))
                wo_v = wo_full[:].rearrange("p (hc w) -> p hc w", hc=H)
                for t8 in range(8):
                    b, tt = t8 // 2, t8 % 2
                    ot_v = ot_sb[b][:].rearrange("p (hc t) -> p hc t", hc=H)
                    for ecol in range(4):
                        ps = wo_ps.tile([P, 512], F32, tag="ps")
                        for hp in range(H // 2):
                            nc.tensor.matmul(
                                ps[:],
                                ot_v[:, 2 * hp:2 * hp + 2, P * tt:P * (tt + 1)],
                                wo_v[:, 2 * hp:2 * hp + 2,
                                     512 * ecol:512 * (ecol + 1)],
                                start=(hp == 0), stop=(hp == H // 2 - 1),
                                perf_mode=DR)
                        nc.vector.scalar_tensor_tensor(
                            out=acc[t8][:, 512 * ecol:512 * (ecol + 1)],
                            in0=ps[:], scalar=c_wo[:],
                            in1=acc[t8][:, 512 * ecol:512 * (ecol + 1)],
                            op0=OP.mult, op1=OP.add)
                # FFN rmsnorm + transpose -> y2T [e, tok]
                for t8 in range(8):
                    scr2 = wo_sb.tile([P, E], BF, tag="scr2")
                    ssq2 = wo_sb.tile([P, 1], F32, tag="ssq2")
                    nc.scalar.activation(scr2[:], acc[t8][:], AF.Square,
                                         accum_out=ssq2[:])
                    sq2 = wo_sb.tile([P, 1], F32, tag="sq2")
                    nc.scalar.activation(sq2[:], ssq2[:], AF.Sqrt,
                                         scale=1.0 / E, bias=eps_t[:])
                    s2 = wo_sb.tile([P, 1], F32, tag="s2")
                    nc.vector.reciprocal(s2[:], sq2[:])
                    y2_t = wo_sb.tile([P, E], BF, tag="y2")
                    nc.scalar.activation(y2_t[:], acc[t8][:], AF.Copy,
                                         scale=s2[:])
                    y2T_v = y2T[:].rearrange("p (c t) -> p c t", t=TSL)
                    for gch in range(4):
                        tps = wo_tps.tile([P, 512], BF, tag="tps")
                        for c4 in range(4):
                            c = 4 * gch + c4
                            nc.tensor.transpose(
                                tps[:, P * c4:P * (c4 + 1)],
                                y2_t[:, P * c:P * (c + 1)], ident[:])
                        nc.vector.tensor_copy(
                            y2T_v[:, 4 * gch:4 * gch + 4, P * t8:P * (t8 + 1)],
                            tps[:].rearrange("p (c t) -> p c t", t=P))
                phB.close()

                # ================= phase C: FFN, streamed full weights ======
                phC = ExitStack()
                wf_p = phC.enter_context(tc.tile_pool(name="wf_p", bufs=2))
                hT_p = phC.enter_context(tc.tile_pool(name="hT_p", bufs=1))
                f1_sb = phC.enter_context(tc.tile_pool(name="f1_sb", bufs=2))
                f1_gps = phC.enter_context(tc.tile_pool(name="f1_gps", bufs=2, space="PSUM"))
                f1_lps = phC.enter_context(tc.tile_pool(name="f1_lps", bufs=2, space="PSUM"))
                f2_ps = phC.enter_context(tc.tile_pool(name="f2_ps", bufs=2, space="PSUM"))

                for k in range(FBLK):
                    hT = hT_p.tile([P, FCB * TSL], BF, tag="hT")
                    for fc in range(FCB):
                        wgs = wf_p.tile([P, EC * P], BF, tag="wg")
                        nc.sync.dma_start(
                            out=wgs[:].rearrange("p (e c) -> p e c", e=EC),
                            in_=wg_in[FCB * k + fc].rearrange("e p c -> p e c"))
                        wls = wf_p.tile([P, EC * P], BF, tag="wl")
                        nc.sync.dma_start(
                            out=wls[:].rearrange("p (e c) -> p e c", e=EC),
                            in_=wl_in[FCB * k + fc].rearrange("e p c -> p e c"))
                        for g2 in range(2):   # 512-token groups
                            psg = f1_gps.tile([P, 512], F32, tag="psg")
                            psl = f1_lps.tile([P, 512], F32, tag="psl")
                            for ec in range(EC):
                                mv = y2T[:, TSL * ec + 512 * g2:
                                         TSL * ec + 512 * (g2 + 1)]
                                nc.tensor.matmul(
                                    psg[:], wgs[:, P * ec:P * (ec + 1)], mv,
                                    start=(ec == 0), stop=(ec == EC - 1),
                                    skip_group_check=True)
                                nc.tensor.matmul(
                                    psl[:], wls[:, P * ec:P * (ec + 1)], mv,
                                    start=(ec == 0), stop=(ec == EC - 1),
                                    skip_group_check=True)
                            tmp_g = f1_sb.tile([P, 512], BF, tag="tmpg")
                            nc.scalar.activation(tmp_g[:], psg[:], AF.Gelu)
                            nc.vector.tensor_tensor(
                                out=hT[:, TSL * fc + 512 * g2:
                                       TSL * fc + 512 * (g2 + 1)],
                                in0=tmp_g[:], in1=psl[:], op=OP.mult)
                    # F2: out partial [tok, E] accumulated into acc
                    for ecol in range(4):
                        wos = wf_p.tile([P, FCB * 512], BF, tag="wos")
                        nc.sync.dma_start(
                            out=wos[:].rearrange("p (f w) -> p f w", f=FCB),
                            in_=wout_in[FCB * k:FCB * (k + 1), :,
                                        512 * ecol:512 * (ecol + 1)].rearrange(
                                "f p w -> p f w"))
                        for t8 in range(8):
                            ps2 = f2_ps.tile([P, 512], F32, tag="ps2")
                            for fs in range(FCB):
                                nc.tensor.matmul(
                                    ps2[:],
                                    hT[:, TSL * fs + P * t8: TSL * fs + P * (t8 + 1)],
                                    wos[:, 512 * fs:512 * (fs + 1)],
                                    start=(fs == 0), stop=(fs == FCB - 1))
                            nc.vector.tensor_tensor(
                                out=acc[t8][:, 512 * ecol:512 * (ecol + 1)],
                                in0=ps2[:],
                                in1=acc[t8][:, 512 * ecol:512 * (ecol + 1)],
                                op=OP.add)
                            if k == FBLK - 1 and ecol == 3:
                                nc.sync.dma_start(
                                    out=out_sl[P * t8:P * (t8 + 1), :],
                                    in_=acc[t8][:])
                phC.close()
    nc.compile()
    return nc, names


def _prep_inputs_shared(inputs):
    """Host-side prep of tensors identical on every core."""
    import ml_dtypes
    BF = ml_dtypes.bfloat16
    x = np.ascontiguousarray(
        np.asarray(inputs["inputs"], np.float32).reshape(TOK, E))
    wo = np.asarray(inputs["wo"], np.float32)
    w_gate = np.asarray(inputs["w_gate"], np.float32)
    w_lin = np.asarray(inputs["w_lin"], np.float32)
    w_out = np.asarray(inputs["w_out"], np.float32)
    gamma_attn = np.asarray(inputs["gamma_attn"], np.float32)
    gamma_ffn = np.asarray(inputs["gamma_ffn"], np.float32)
    positions = np.asarray(inputs["positions"])

    F8 = ml_dtypes.float8_e4m3
    wo_r = np.ascontiguousarray((wo.reshape(H, P, E) * 256.0).astype(F8))
    # [E, F] -> [FC, EC, P, P] (fcol-major strips)
    def _gl(w):
        w = (w * gamma_ffn[:, None]).astype(BF)
        return np.ascontiguousarray(
            w.reshape(EC, P, FC, P).transpose(2, 0, 1, 3))
    wg_t = _gl(w_gate)
    wl_t = _gl(w_lin)
    wout_t = np.ascontiguousarray(w_out.reshape(FC, P, E).astype(BF))

    assert np.all(gamma_attn == gamma_attn[0]), \
        "non-uniform gamma_attn needs full-width rope tables"
    half = DH // 2
    inv_freq = (1.0 / (10000.0 ** (np.arange(half, dtype=np.float32) / half))
                ).astype(np.float32)
    ang = positions.astype(np.float32)[:, None] * inv_freq[None, :]
    g0 = float(gamma_attn[0])
    cos = (np.cos(ang) * g0).astype(BF)
    sin = (np.sin(ang) * g0).astype(BF)

    k_i = np.arange(P)[:, None]
    q_i = np.arange(512)[None, :]
    msk = np.stack([(P * m + k_i <= q_i) for m in range(4)]).astype(F8)

    return {
        "wo": wo_r, "wg": wg_t, "wl": wl_t, "wout": wout_t,
        "cos": cos, "sin": sin, "mask": msk,
        "onec": np.ones((P, 32), F8),
        "oner": np.ones((1, P), np.float32),
    }, x


def _prep_inputs_core(inputs, x_f32, r):
    """Per-core tensors: head-sharded QKV weights + owned-token x slice."""
    import ml_dtypes
    F8 = ml_dtypes.float8_e4m3
    wq = np.asarray(inputs["wq"], np.float32) / np.sqrt(np.float32(DH))
    wk = np.asarray(inputs["wk"], np.float32)
    wv = np.asarray(inputs["wv"], np.float32)
    h0 = HLOC * r

    def _slice_qkv(w):   # [E, H, DH] -> [EC, P, HLOC*DH]
        return w[:, h0:h0 + HLOC, :].reshape(EC, P, HLOC * DH)

    wqkv = (np.concatenate([_slice_qkv(wq), _slice_qkv(wk), _slice_qkv(wv)],
                           axis=2) * 512.0).astype(F8)
    # tokens owned by core r: 256 from each batch
    xsl = np.ascontiguousarray(
        x_f32.reshape(B, NCORES, TB, E)[:, r].reshape(TSL, E))
    import ml_dtypes as _md
    xbf = np.ascontiguousarray(xsl.astype(_md.bfloat16))
    return {"wqkv": np.ascontiguousarray(wqkv), "xsl": xsl, "x": xbf}


def _run(inputs, trace=False):
    from concourse.bass_utils import run_bass_kernel_spmd

    if "nc" not in _CACHE:
        _CACHE["nc"], _CACHE["names"] = _build()
    nc, names = _CACHE["nc"], _CACHE["names"]

    shared, x_f32 = _prep_inputs_shared(inputs)
    cos_full = shared.pop("cos")
    sin_full = shared.pop("sin")
    in_maps = []
    for r in range(NCORES):
        prep = dict(shared)
        prep["cos"] = np.ascontiguousarray(cos_full[TB * r:TB * (r + 1)])
        prep["sin"] = np.ascontiguousarray(sin_full[TB * r:TB * (r + 1)])
        prep.update(_prep_inputs_core(inputs, x_f32, r))
        in_maps.append({names[k]: v for k, v in prep.items()})

    res = run_bass_kernel_spmd(nc, in_maps, core_ids=list(range(NCORES)),
                               trace=trace)
    out = np.empty((B, NCORES, TB, E), np.float32)
    for r in range(NCORES):
        out[:, r] = res.results[r][names["out"]].reshape(B, TB, E)
    return out.reshape(B, S, E), res


def kernel(**inputs) -> np.ndarray:
    return _run(inputs)[0]


# revision 32
# speedup vs baseline: 1.1578x; 1.0155x over previous
"""Trainium2 Bass kernel for a dense transformer decoder block.

Tensor-parallel over 8 NeuronCores, bf16 matmuls (fp32 accumulation):
  Phase A: heads sharded (2/core). norm+rope+QKV+causal attention over all
           tokens; per-batch AllToAll redistributes attention output to
           token owners (each core owns 256 tokens of each batch).
  Phase B: WO projection + residual into a persistent fp32 accumulator,
           then FFN rmsnorm -> y2^T.
  Phase C: FFN over the full hidden dim with streamed bf16 weights
           (no collectives); F2 accumulates into the fp32 accumulator.
"""
import sys

if '/opt/trn_rl_repo' not in sys.path:
    sys.path.insert(0, '/opt/trn_rl_repo')

import numpy as np
from contextlib import ExitStack

B, S, E, H, DH, F = 4, 2048, 2048, 16, 128, 8192
P = 128
NCORES = 8
HLOC = H // NCORES          # 2 heads per core
TOK = B * S                 # 8192 tokens
TSL = TOK // NCORES         # 1024 tokens per core (256 from each batch)
TB = TSL // B               # 256 tokens per (core, batch)
EC = E // P                 # 16 embedding chunks
FC = F // P                 # 64 FFN col chunks
FBLK = 4                    # outer F blocks
FCB = FC // FBLK            # 16 col chunks per block
EPS = 1e-5
ATB = 256                   # phase-A token block
NTB = S // ATB              # 8 blocks per batch

_CACHE = {}


def _build():
    import concourse.bacc as bacc
    import concourse.mybir as mybir
    import concourse.tile as tile
    import concourse.tile_utils as tile_utils
    from concourse.masks import make_identity

    tile_utils.max_sbuf_usage = 204 * 1024

    F32 = mybir.dt.float32
    F32R = mybir.dt.float32r
    BF = mybir.dt.bfloat16
    F8 = mybir.dt.float8e4
    DR = mybir.MatmulPerfMode.DoubleRow
    AF = mybir.ActivationFunctionType
    OP = mybir.AluOpType

    nc = bacc.Bacc(None, target_bir_lowering=False)
    names = {}

    with tile.TileContext(nc) as tc:
        with tc.tile_pool(name="dram", bufs=1, space="DRAM") as dram:
            # ---- external inputs ----
            xbf_in = dram.tile([TSL, E], BF, kind="ExternalInput")
            xsl_in = dram.tile([TSL, E], F32, kind="ExternalInput")
            wqkv_in = dram.tile([EC, P, 6 * P], F8, kind="ExternalInput")
            wo_in = dram.tile([H, P, E], F8, kind="ExternalInput")
            wg_in = dram.tile([FC, EC, P, P], BF, kind="ExternalInput")
            wl_in = dram.tile([FC, EC, P, P], BF, kind="ExternalInput")
            wout_in = dram.tile([FC, P, E], BF, kind="ExternalInput")
            cos_in = dram.tile([TB, 64], BF, kind="ExternalInput")
            sin_in = dram.tile([TB, 64], BF, kind="ExternalInput")
            mask_in = dram.tile([4, P, 512], F8, kind="ExternalInput")
            onec_in = dram.tile([P, 32], F8, kind="ExternalInput")
            oner_in = dram.tile([1, P], F32R, kind="ExternalInput")
            out_sl = dram.tile([TSL, E], F32, kind="ExternalOutput")
            names.update(
                x=xbf_in.name, xsl=xsl_in.name, wqkv=wqkv_in.name, wo=wo_in.name,
                wg=wg_in.name, wl=wl_in.name, wout=wout_in.name,
                cos=cos_in.name, sin=sin_in.name, mask=mask_in.name,
                onec=onec_in.name, oner=oner_in.name, out=out_sl.name)

            # ---- internal DRAM: per-batch AllToAll bounce ----
            a2a_in = [dram.tile([NCORES * HLOC * P, TB], F8, name=f"a2ai{b}")
                      for b in range(B)]
            agy_in = [dram.tile([E, TB], F8, name=f"agyi{b}")
                      for b in range(B)]
            agy_out = [dram.tile([NCORES * E, TB], F8, name=f"agyo{b}",
                                 addr_space="Shared")
                       for b in range(B)]
            a2a_out = [dram.tile([NCORES * HLOC * P, TB], F8,
                                 name=f"a2ao{b}")
                       for b in range(B)]

            RG = [list(range(NCORES))]

            with tc.tile_pool(name="cst", bufs=1) as cst, \
                 tc.tile_pool(name="acc_p", bufs=1) as acc_p, \
                 tc.tile_pool(name="y2T_p", bufs=1) as y2T_p:
                ident = cst.tile([P, P], BF)
                make_identity(nc, ident[:])
                eps_t = cst.tile([P, 1], F32)
                nc.gpsimd.memset(eps_t[:], EPS)
                neg1 = cst.tile([P, 1], F32)
                nc.gpsimd.memset(neg1[:], -1.0)
                c_wo = cst.tile([P, 1], F32)
                nc.gpsimd.memset(c_wo[:], 1.0 / 4096.0)

                # phase-B staging pools opened below the phase-A pools on the
                # pool stack (LIFO) with fresh SBUF addresses, so their DMAs
                # aren't WAR-gated behind attention's last reads.
                phB = ExitStack()
                ot_p = phB.enter_context(tc.tile_pool(name="ot_p", bufs=1))
                wo_p = phB.enter_context(tc.tile_pool(name="wo_p", bufs=1))
                wof0 = wo_p.tile([P, H * 1024], F8, tag="wof0")
                nc.gpsimd.dma_start(
                    out=wof0[:].rearrange("p (hc w) -> p hc w", hc=H),
                    in_=wo_in[:, :, 0:1024].rearrange("hc p w -> p hc w"))
                ot_sb = []

                # ================= phase A: norm+rope+QKV+attention =========
                stgA = ExitStack()
                wqkv_p = stgA.enter_context(tc.tile_pool(name="wqkv_p", bufs=1))
                tabs = stgA.enter_context(tc.tile_pool(name="tabs", bufs=1))
                ytb_p = stgA.enter_context(tc.tile_pool(name="ytb", bufs=2))
                qkvb_p = stgA.enter_context(tc.tile_pool(name="qkvb", bufs=1))
                st_sb = stgA.enter_context(tc.tile_pool(name="st_sb", bufs=2))
                scr_p = stgA.enter_context(tc.tile_pool(name="scr_p", bufs=1))
                st_ps = stgA.enter_context(tc.tile_pool(name="st_ps", bufs=2, space="PSUM"))
                qkv_ps = stgA.enter_context(tc.tile_pool(name="qkv_ps", bufs=1, space="PSUM"))
                at_s_ps = stgA.enter_context(tc.tile_pool(name="at_s_ps", bufs=2, space="PSUM"))
                at_o_ps = stgA.enter_context(tc.tile_pool(name="at_o_ps", bufs=1, space="PSUM"))
                at_db_ps = stgA.enter_context(tc.tile_pool(name="at_db_ps", bufs=1, space="PSUM"))
                at_sb = stgA.enter_context(tc.tile_pool(name="at_sb", bufs=2))

                wqkv_sb = wqkv_p.tile([P, EC * 6 * P], F8)
                nc.sync.dma_start(
                    out=wqkv_sb[:].rearrange("p (e c) -> p e c", e=EC),
                    in_=wqkv_in[:].rearrange("e p c -> p e c"))
                ones_col = tabs.tile([P, 32], F8)
                nc.sync.dma_start(out=ones_col[:], in_=onec_in[:])
                ones_row = tabs.tile([1, P], F32R)
                nc.sync.dma_start(out=ones_row[:], in_=oner_in[:])
                masks = tabs.tile([P, 4 * 512], F8)
                nc.sync.dma_start(
                    out=masks[:].rearrange("p (m w) -> p m w", m=4),
                    in_=mask_in[:].rearrange("m p w -> p m w"))
                # rope tables for own 256 positions -> sbuf [pos%128, (blk, j)]
                cos_all = tabs.tile([P, 2 * 64], BF)
                sin_all = tabs.tile([P, 2 * 64], BF)
                nc.sync.dma_start(
                    out=cos_all[:].rearrange("p (r j) -> p r j", r=2),
                    in_=cos_in[:].rearrange("(r p) j -> p r j", p=P))
                nc.sync.dma_start(
                    out=sin_all[:].rearrange("p (r j) -> p r j", r=2),
                    in_=sin_in[:].rearrange("(r p) j -> p r j", p=P))

                def prep_batch(pb):
                    """norm+rope+transpose own 256 tokens of batch pb,
                    write yT slice to agy_in[pb], then AllGather."""
                    for tt in range(2):
                        row = TB * pb + P * tt
                        x_t = st_sb.tile([P, E], BF, tag="x")
                        nc.sync.dma_start(out=x_t[:], in_=xbf_in[row:row + P, :])
                        y_t = st_sb.tile([P, E], BF, tag="y")
                        t1 = st_sb.tile([P, E], BF, tag="t1")
                        ssq = st_sb.tile([P, 1], F32, tag="ssq")
                        nc.scalar.activation(y_t[:], x_t[:], AF.Square,
                                             accum_out=ssq[:])
                        sq = st_sb.tile([P, 1], F32, tag="sq")
                        nc.scalar.activation(sq[:], ssq[:], AF.Sqrt,
                                             scale=1.0 / E, bias=eps_t[:])
                        s_t = st_sb.tile([P, 1], F32, tag="s")
                        nc.vector.reciprocal(s_t[:], sq[:])
                        xr = x_t[:].rearrange("p (c two h) -> p c two h", two=2, h=64)
                        yr = y_t[:].rearrange("p (c two h) -> p c two h", two=2, h=64)
                        tr = t1[:].rearrange("p (c two h) -> p c two h", two=2, h=64)
                        cb = cos_all[:, 64 * tt:64 * (tt + 1)].rearrange(
                            "p (o j) -> p o j", o=1).broadcast_to([P, EC, 64])
                        sb_ = sin_all[:, 64 * tt:64 * (tt + 1)].rearrange(
                            "p (o j) -> p o j", o=1).broadcast_to([P, EC, 64])
                        nc.vector.scalar_tensor_tensor(
                            out=tr[:, :, 0], in0=xr[:, :, 1], scalar=s_t[:],
                            in1=sb_, op0=OP.mult, op1=OP.mult)
                        nc.vector.scalar_tensor_tensor(
                            out=yr[:, :, 0], in0=xr[:, :, 0], scalar=s_t[:],
                            in1=cb, op0=OP.mult, op1=OP.mult)
                        nc.vector.tensor_tensor(
                            out=yr[:, :, 0], in0=yr[:, :, 0], in1=tr[:, :, 0],
                            op=OP.subtract)
                        nc.vector.scalar_tensor_tensor(
                            out=tr[:, :, 1], in0=xr[:, :, 0], scalar=s_t[:],
                            in1=sb_, op0=OP.mult, op1=OP.mult)
                        nc.vector.scalar_tensor_tensor(
                            out=yr[:, :, 1], in0=xr[:, :, 1], scalar=s_t[:],
                            in1=cb, op0=OP.mult, op1=OP.mult)
                        nc.vector.tensor_tensor(
                            out=yr[:, :, 1], in0=yr[:, :, 1], in1=tr[:, :, 1],
                            op=OP.add)
                        yTo = st_sb.tile([P, EC * P], F8, tag="yTo")
                        yTo_v = yTo[:].rearrange("p (c t) -> p c t", t=P)
                        for gch in range(4):
                            tps = st_ps.tile([P, 512], BF, tag="tps")
                            for c4 in range(4):
                                c = 4 * gch + c4
                                nc.tensor.transpose(
                                    tps[:, P * c4:P * (c4 + 1)],
                                    y_t[:, P * c:P * (c + 1)], ident[:])
                            nc.scalar.activation(
                                yTo_v[:, 4 * gch:4 * gch + 4, :],
                                tps[:].rearrange("p (c t) -> p c t", t=P),
                                AF.Copy, scale=16.0)
                        nc.sync.dma_start(
                            out=agy_in[pb][:].rearrange(
                                "(ec p) t -> p ec t", p=P)[:, :, P * tt:P * (tt + 1)],
                            in_=yTo_v)
                    nc.gpsimd.collective_compute(
                        "AllGather", OP.bypass, replica_groups=RG,
                        ins=[agy_in[pb][:]], outs=[agy_out[pb][:]])

                prep_batch(0)
                for b in range(B):
                    qt_b = qkvb_p.tile([P, HLOC * S], BF, tag="qt")
                    kt_b = qkvb_p.tile([P, HLOC * S], BF, tag="kt")
                    v_b = qkvb_p.tile([P, HLOC * S], F8, tag="vb")
                    agy_v = agy_out[b][:].rearrange(
                        "(s ec p) t -> s ec p t", s=NCORES, p=P)
                    for tb in range(NTB):    # 256-token blocks (= src core tb)
                        yT = ytb_p.tile([P, EC * ATB], F8, tag="yT")
                        nc.sync.dma_start(
                            out=yT[:].rearrange("p (e t) -> p e t", e=EC),
                            in_=agy_v[tb].rearrange("ec p t -> p ec t"))
                        # QKV matmuls (6 groups of 128 cols)
                        for half in range(2):
                            for g3 in range(3):
                                g = 3 * half + g3
                                pq = qkv_ps.tile([P, ATB], F32, tag=f"pq{g3 % 2}")
                                wv_v = wqkv_sb[:].rearrange(
                                    "p (e c) -> p e c", e=EC)
                                yT_v = yT[:].rearrange("p (e t) -> p e t", e=EC)
                                for ep in range(EC // 2):
                                    nc.tensor.matmul(
                                        pq[:],
                                        wv_v[:, 2 * ep:2 * ep + 2, P * g:P * (g + 1)],
                                        yT_v[:, 2 * ep:2 * ep + 2, :],
                                        start=(ep == 0), stop=(ep == EC // 2 - 1),
                                        perf_mode=DR)
                                h = g % 2
                                col = S * h + ATB * tb
                                if g < 2:      # Q heads (scale folded host-side)
                                    nc.scalar.activation(qt_b[:, col:col + ATB],
                                                         pq[:], AF.Copy,
                                                         scale=1.0 / 8192.0)
                                elif g < 4:    # K heads
                                    nc.scalar.activation(kt_b[:, col:col + ATB],
                                                         pq[:], AF.Copy,
                                                         scale=1.0 / 8192.0)
                                else:          # V heads -> transpose to [tok, DH]
                                    vt_tmp = st_sb.tile([P, ATB], BF, tag="vt")
                                    nc.scalar.activation(vt_tmp[:], pq[:], AF.Copy,
                                                         scale=1.0 / 512.0)
                                    tpv = st_ps.tile([P, 512], BF, tag="tps")
                                    for j in range(ATB // P):
                                        nc.tensor.transpose(
                                            tpv[:, P * j:P * (j + 1)],
                                            vt_tmp[:, P * j:P * (j + 1)], ident[:])
                                    nc.vector.tensor_copy(v_b[:, col:col + ATB],
                                                          tpv[:, 0:ATB])
                    if b + 1 < B:
                        prep_batch(b + 1)
                    # ---- attention for batch b, both local heads ----
                    for h in range(HLOC):
                        hs = S * h
                        for qb in range(4):
                            nk = 4 * qb + 4
                            ps_o = at_o_ps.tile([P, 512], F32, tag="pso")
                            ps_d = at_db_ps.tile([1, 512], F32, tag="db")
                            nkp = nk // 2
                            for ktp in range(nkp):
                                probs = at_sb.tile([P, 1024], F8, tag="probs")
                                for sub in range(2):
                                    kt = 2 * ktp + sub
                                    ps_s = at_s_ps.tile([P, 512], F32, tag="pss")
                                    nc.tensor.matmul(
                                        ps_s[:],
                                        kt_b[:, hs + P * kt: hs + P * (kt + 1)],
                                        qt_b[:, hs + 512 * qb: hs + 512 * (qb + 1)],
                                        start=True, stop=True)
                                    pr = probs[:, 512 * sub:512 * (sub + 1)]
                                    nc.scalar.activation(pr, ps_s[:], AF.Exp,
                                                         bias=neg1[:])
                                    if kt >= 4 * qb:
                                        m = kt - 4 * qb
                                        nc.vector.tensor_tensor(
                                            out=pr, in0=pr,
                                            in1=masks[:, 512 * m:512 * (m + 1)],
                                            op=OP.mult)
                                pr2 = probs[:].rearrange(
                                    "p (two t) -> p two t", two=2)
                                nc.tensor.matmul(
                                    ps_o[:],
                                    v_b[:, hs + 2 * P * ktp: hs + 2 * P * (ktp + 1)].rearrange(
                                        "p (two t) -> p two t", two=2),
                                    pr2,
                                    start=(ktp == 0), stop=(ktp == nkp - 1),
                                    skip_group_check=True, perf_mode=DR)
                                nc.tensor.matmul(
                                    ps_d[:],
                                    ones_col[:].rearrange(
                                        "p (two t) -> p two t", two=2)[:, :, 0:1],
                                    pr2,
                                    start=(ktp == 0), stop=(ktp == nkp - 1),
                                    skip_group_check=True, perf_mode=DR)
                            rd = scr_p.tile([1, 512], F32R, tag="rd")
                            with nc.allow_low_precision(reason="softmax denom"):
                                nc.vector.reciprocal(rd[:], ps_d[:])
                            ps_b = at_db_ps.tile([P, 512], F32, tag="db")
                            nc.tensor.matmul(ps_b[:], ones_row[:], rd[:],
                                             start=True, stop=True)
                            osb = at_sb.tile([P, 512], BF, tag="osb")
                            nc.scalar.copy(osb[:], ps_o[:])
                            ot = at_sb.tile([P, 512], F8, tag="ot")
                            nc.vector.tensor_tensor(out=ot[:], in0=osb[:],
                                                    in1=ps_b[:], op=OP.mult)
                            # tokens 512*qb..512*qb+512 of batch b ->
                            # dest cores 2qb (first 256) and 2qb+1
                            a2a_v = a2a_in[b][:].rearrange(
                                "(d r) t -> d r t", d=NCORES)
                            for m in range(2):
                                nc.sync.dma_start(
                                    out=a2a_v[2 * qb + m, P * h:P * (h + 1), :],
                                    in_=ot[:, TB * m:TB * (m + 1)])
                    nc.gpsimd.collective_compute(
                        "AllToAll", OP.bypass, replica_groups=RG,
                        ins=[a2a_in[b][:]], outs=[a2a_out[b][:]])
                    o8 = ot_p.tile([P, H * TB], F8, name=f"otb8{b}")
                    nc.gpsimd.dma_start(
                        out=o8[:].rearrange("p (hc t) -> p hc t", hc=H),
                        in_=a2a_out[b][:].rearrange("(hc p) t -> p hc t", p=P))
                    ot_sb.append(o8)
                stgA.close()

                # ================= phase B: WO + residual + FFN norm ========
                # fp32 accumulator tiles hold x + attn@wo, later += FFN out.
                acc = []
                for t8 in range(8):
                    a = acc_p.tile([P, E], F32, name=f"acc{t8}")
                    nc.sync.dma_start(out=a[:], in_=xsl_in[P * t8:P * (t8 + 1), :])
                    acc.append(a)
                y2T = y2T_p.tile([P, EC * TSL], BF)

                wo_sb = phB.enter_context(tc.tile_pool(name="wo_sb", bufs=2))
                wo_ps = phB.enter_context(tc.tile_pool(name="wo_ps", bufs=2, space="PSUM"))
                wo_tps = phB.enter_context(tc.tile_pool(name="wo_tps", bufs=2, space="PSUM"))
                wo_p2 = phB.enter_context(tc.tile_pool(name="wo_p2", bufs=1))
                wof1 = wo_p2.tile([P, H * 1024], F8, tag="wof1")
                nc.gpsimd.dma_start(
                    out=wof1[:].rearrange("p (hc w) -> p hc w", hc=H),
                    in_=wo_in[:, :, 1024:2048].rearrange("hc p w -> p hc w"))
                wo_halves = [wof0[:].rearrange("p (hc w) -> p hc w", hc=H),
                             wof1[:].rearrange("p (hc w) -> p hc w", hc=H)]
                for t8 in range(8):
                    b, tt = t8 // 2, t8 % 2
                    ot_v = ot_sb[b][:].rearrange("p (hc t) -> p hc t", hc=H)
                    for ecol in range(4):
                        ps = wo_ps.tile([P, 512], F32, tag="ps")
                        wo_h = wo_halves[ecol // 2]
                        ec2 = ecol % 2
                        for hp in range(H // 2):
                            nc.tensor.matmul(
                                ps[:],
                                ot_v[:, 2 * hp:2 * hp + 2, P * tt:P * (tt + 1)],
                                wo_h[:, 2 * hp:2 * hp + 2,
                                     512 * ec2:512 * (ec2 + 1)],
                                start=(hp == 0), stop=(hp == H // 2 - 1),
                                perf_mode=DR)
                        nc.vector.scalar_tensor_tensor(
                            out=acc[t8][:, 512 * ecol:512 * (ecol + 1)],
                            in0=ps[:], scalar=c_wo[:],
                            in1=acc[t8][:, 512 * ecol:512 * (ecol + 1)],
                            op0=OP.mult, op1=OP.add)
                # FFN rmsnorm + transpose -> y2T [e, tok]
                for t8 in range(8):
                    scr2 = wo_sb.tile([P, E], BF, tag="scr2")
                    ssq2 = wo_sb.tile([P, 1], F32, tag="ssq2")
                    nc.scalar.activation(scr2[:], acc[t8][:], AF.Square,
                                         accum_out=ssq2[:])
                    sq2 = wo_sb.tile([P, 1], F32, tag="sq2")
                    nc.scalar.activation(sq2[:], ssq2[:], AF.Sqrt,
                                         scale=1.0 / E, bias=eps_t[:])
                    s2 = wo_sb.tile([P, 1], F32, tag="s2")
                    nc.vector.reciprocal(s2[:], sq2[:])
                    y2_t = wo_sb.tile([P, E], BF, tag="y2")
                    nc.scalar.activation(y2_t[:], acc[t8][:], AF.Copy,
                                         scale=s2[:])
                    y2T_v = y2T[:].rearrange("p (c t) -> p c t", t=TSL)
                    for gch in range(4):
                        tps = wo_tps.tile([P, 512], BF, tag="tps")
                        for c4 in range(4):
                            c = 4 * gch + c4
                            nc.tensor.transpose(
                                tps[:, P * c4:P * (c4 + 1)],
                                y2_t[:, P * c:P * (c + 1)], ident[:])
                        nc.vector.tensor_copy(
                            y2T_v[:, 4 * gch:4 * gch + 4, P * t8:P * (t8 + 1)],
                            tps[:].rearrange("p (c t) -> p c t", t=P))
                phB.close()

                # ================= phase C: FFN, streamed full weights ======
                phC = ExitStack()
                wf_p = phC.enter_context(tc.tile_pool(name="wf_p", bufs=2))
                hT_p = phC.enter_context(tc.tile_pool(name="hT_p", bufs=1))
                f1_sb = phC.enter_context(tc.tile_pool(name="f1_sb", bufs=2))
                f1_gps = phC.enter_context(tc.tile_pool(name="f1_gps", bufs=2, space="PSUM"))
                f1_lps = phC.enter_context(tc.tile_pool(name="f1_lps", bufs=2, space="PSUM"))
                f2_ps = phC.enter_context(tc.tile_pool(name="f2_ps", bufs=2, space="PSUM"))

                for k in range(FBLK):
                    hT = hT_p.tile([P, FCB * TSL], BF, tag="hT")
                    for fc in range(FCB):
                        wgs = wf_p.tile([P, EC * P], BF, tag="wg")
                        nc.sync.dma_start(
                            out=wgs[:].rearrange("p (e c) -> p e c", e=EC),
                            in_=wg_in[FCB * k + fc].rearrange("e p c -> p e c"))
                        wls = wf_p.tile([P, EC * P], BF, tag="wl")
                        nc.sync.dma_start(
                            out=wls[:].rearrange("p (e c) -> p e c", e=EC),
                            in_=wl_in[FCB * k + fc].rearrange("e p c -> p e c"))
                        for g2 in range(2):   # 512-token groups
                            psg = f1_gps.tile([P, 512], F32, tag="psg")
                            psl = f1_lps.tile([P, 512], F32, tag="psl")
                            for ec in range(EC):
                                mv = y2T[:, TSL * ec + 512 * g2:
                                         TSL * ec + 512 * (g2 + 1)]
                                nc.tensor.matmul(
                                    psg[:], wgs[:, P * ec:P * (ec + 1)], mv,
                                    start=(ec == 0), stop=(ec == EC - 1),
                                    skip_group_check=True)
                                nc.tensor.matmul(
                                    psl[:], wls[:, P * ec:P * (ec + 1)], mv,
                                    start=(ec == 0), stop=(ec == EC - 1),
                                    skip_group_check=True)
                            tmp_g = f1_sb.tile([P, 512], BF, tag="tmpg")
                            nc.scalar.activation(tmp_g[:], psg[:], AF.Gelu)
                            nc.vector.tensor_tensor(
                                out=hT[:, TSL * fc + 512 * g2:
                                       TSL * fc + 512 * (g2 + 1)],
                                in0=tmp_g[:], in1=psl[:], op=OP.mult)
                    # F2: out partial [tok, E] accumulated into acc
                    for ecol in range(4):
                        wos = wf_p.tile([P, FCB * 512], BF, tag="wos")
                        nc.sync.dma_start(
                            out=wos[:].rearrange("p (f w) -> p f w", f=FCB),
                            in_=wout_in[FCB * k:FCB * (k + 1), :,
                                        512 * ecol:512 * (ecol + 1)].rearrange(
                                "f p w -> p f w"))
                        for t8 in range(8):
                            ps2 = f2_ps.tile([P, 512], F32, tag="ps2")
                            for fs in range(FCB):
                                nc.tensor.matmul(
                                    ps2[:],
                                    hT[:, TSL * fs + P * t8: TSL * fs + P * (t8 + 1)],
                                    wos[:, 512 * fs:512 * (fs + 1)],
                                    start=(fs == 0), stop=(fs == FCB - 1))
                            nc.vector.tensor_tensor(
                                out=acc[t8][:, 512 * ecol:512 * (ecol + 1)],
                                in0=ps2[:],
                                in1=acc[t8][:, 512 * ecol:512 * (ecol + 1)],
                                op=OP.add)
                            if k == FBLK - 1 and ecol == 3:
                                nc.sync.dma_start(
                                    out=out_sl[P * t8:P * (t8 + 1), :],
                                    in_=acc[t8][:])
                phC.close()
    nc.compile()
    return nc, names


def _prep_inputs_shared(inputs):
    """Host-side prep of tensors identical on every core."""
    import ml_dtypes
    BF = ml_dtypes.bfloat16
    x = np.ascontiguousarray(
        np.asarray(inputs["inputs"], np.float32).reshape(TOK, E))
    wo = np.asarray(inputs["wo"], np.float32)
    w_gate = np.asarray(inputs["w_gate"], np.float32)
    w_lin = np.asarray(inputs["w_lin"], np.float32)
    w_out = np.asarray(inputs["w_out"], np.float32)
    gamma_attn = np.asarray(inputs["gamma_attn"], np.float32)
    gamma_ffn = np.asarray(inputs["gamma_ffn"], np.float32)
    positions = np.asarray(inputs["positions"])

    F8 = ml_dtypes.float8_e4m3
    wo_r = np.ascontiguousarray((wo.reshape(H, P, E) * 256.0).astype(F8))
    # [E, F] -> [FC, EC, P, P] (fcol-major strips)
    def _gl(w):
        w = (w * gamma_ffn[:, None]).astype(BF)
        return np.ascontiguousarray(
            w.reshape(EC, P, FC, P).transpose(2, 0, 1, 3))
    wg_t = _gl(w_gate)
    wl_t = _gl(w_lin)
    wout_t = np.ascontiguousarray(w_out.reshape(FC, P, E).astype(BF))

    assert np.all(gamma_attn == gamma_attn[0]), \
        "non-uniform gamma_attn needs full-width rope tables"
    half = DH // 2
    inv_freq = (1.0 / (10000.0 ** (np.arange(half, dtype=np.float32) / half))
                ).astype(np.float32)
    ang = positions.astype(np.float32)[:, None] * inv_freq[None, :]
    g0 = float(gamma_attn[0])
    cos = (np.cos(ang) * g0).astype(BF)
    sin = (np.sin(ang) * g0).astype(BF)

    k_i = np.arange(P)[:, None]
    q_i = np.arange(512)[None, :]
    msk = np.stack([(P * m + k_i <= q_i) for m in range(4)]).astype(F8)

    return {
        "wo": wo_r, "wg": wg_t, "wl": wl_t, "wout": wout_t,
        "cos": cos, "sin": sin, "mask": msk,
        "onec": np.ones((P, 32), F8),
        "oner": np.ones((1, P), np.float32),
    }, x


def _prep_inputs_core(inputs, x_f32, r):
    """Per-core tensors: head-sharded QKV weights + owned-token x slice."""
    import ml_dtypes
    F8 = ml_dtypes.float8_e4m3
    wq = np.asarray(inputs["wq"], np.float32) / np.sqrt(np.float32(DH))
    wk = np.asarray(inputs["wk"], np.float32)
    wv = np.asarray(inputs["wv"], np.float32)
    h0 = HLOC * r

    def _slice_qkv(w):   # [E, H, DH] -> [EC, P, HLOC*DH]
        return w[:, h0:h0 + HLOC, :].reshape(EC, P, HLOC * DH)

    wqkv = (np.concatenate([_slice_qkv(wq), _slice_qkv(wk), _slice_qkv(wv)],
                           axis=2) * 512.0).astype(F8)
    # tokens owned by core r: 256 from each batch
    xsl = np.ascontiguousarray(
        x_f32.reshape(B, NCORES, TB, E)[:, r].reshape(TSL, E))
    import ml_dtypes as _md
    xbf = np.ascontiguousarray(xsl.astype(_md.bfloat16))
    return {"wqkv": np.ascontiguousarray(wqkv), "xsl": xsl, "x": xbf}


def _run(inputs, trace=False):
    from concourse.bass_utils import run_bass_kernel_spmd

    if "nc" not in _CACHE:
        _CACHE["nc"], _CACHE["names"] = _build()
    nc, names = _CACHE["nc"], _CACHE["names"]

    shared, x_f32 = _prep_inputs_shared(inputs)
    cos_full = shared.pop("cos")
    sin_full = shared.pop("sin")
    in_maps = []
    for r in range(NCORES):
        prep = dict(shared)
        prep["cos"] = np.ascontiguousarray(cos_full[TB * r:TB * (r + 1)])
        prep["sin"] = np.ascontiguousarray(sin_full[TB * r:TB * (r + 1)])
        prep.update(_prep_inputs_core(inputs, x_f32, r))
        in_maps.append({names[k]: v for k, v in prep.items()})

    res = run_bass_kernel_spmd(nc, in_maps, core_ids=list(range(NCORES)),
                               trace=trace)
    out = np.empty((B, NCORES, TB, E), np.float32)
    for r in range(NCORES):
        out[:, r] = res.results[r][names["out"]].reshape(B, TB, E)
    return out.reshape(B, S, E), res


def kernel(**inputs) -> np.ndarray:
    return _run(inputs)[0]


# revision 33
# speedup vs baseline: 1.1686x; 1.0093x over previous
"""Trainium2 Bass kernel for a dense transformer decoder block.

Tensor-parallel over 8 NeuronCores, bf16 matmuls (fp32 accumulation):
  Phase A: heads sharded (2/core). norm+rope+QKV+causal attention over all
           tokens; per-batch AllToAll redistributes attention output to
           token owners (each core owns 256 tokens of each batch).
  Phase B: WO projection + residual into a persistent fp32 accumulator,
           then FFN rmsnorm -> y2^T.
  Phase C: FFN over the full hidden dim with streamed bf16 weights
           (no collectives); F2 accumulates into the fp32 accumulator.
"""
import sys

if '/opt/trn_rl_repo' not in sys.path:
    sys.path.insert(0, '/opt/trn_rl_repo')

import numpy as np
from contextlib import ExitStack

B, S, E, H, DH, F = 4, 2048, 2048, 16, 128, 8192
P = 128
NCORES = 8
HLOC = H // NCORES          # 2 heads per core
TOK = B * S                 # 8192 tokens
TSL = TOK // NCORES         # 1024 tokens per core (256 from each batch)
TB = TSL // B               # 256 tokens per (core, batch)
EC = E // P                 # 16 embedding chunks
FC = F // P                 # 64 FFN col chunks
FBLK = 4                    # outer F blocks
FCB = FC // FBLK            # 16 col chunks per block
EPS = 1e-5
ATB = 256                   # phase-A token block
NTB = S // ATB              # 8 blocks per batch

_CACHE = {}


def _build():
    import concourse.bacc as bacc
    import concourse.mybir as mybir
    import concourse.tile as tile
    import concourse.tile_utils as tile_utils
    from concourse.masks import make_identity

    tile_utils.max_sbuf_usage = 204 * 1024

    F32 = mybir.dt.float32
    F32R = mybir.dt.float32r
    BF = mybir.dt.bfloat16
    F8 = mybir.dt.float8e4
    DR = mybir.MatmulPerfMode.DoubleRow
    AF = mybir.ActivationFunctionType
    OP = mybir.AluOpType

    nc = bacc.Bacc(None, target_bir_lowering=False)
    names = {}

    with tile.TileContext(nc) as tc:
        with tc.tile_pool(name="dram", bufs=1, space="DRAM") as dram:
            # ---- external inputs ----
            xbf_in = dram.tile([TSL, E], BF, kind="ExternalInput")
            xsl_in = dram.tile([TSL, E], F32, kind="ExternalInput")
            wqkv_in = dram.tile([EC, P, 6 * P], F8, kind="ExternalInput")
            wo_in = dram.tile([H, P, E], F8, kind="ExternalInput")
            wg_in = dram.tile([FC, EC, P, P], BF, kind="ExternalInput")
            wl_in = dram.tile([FC, EC, P, P], BF, kind="ExternalInput")
            wout_in = dram.tile([FC, P, E], BF, kind="ExternalInput")
            cos_in = dram.tile([TB, 64], BF, kind="ExternalInput")
            sin_in = dram.tile([TB, 64], BF, kind="ExternalInput")
            mask_in = dram.tile([4, P, 512], F8, kind="ExternalInput")
            onec_in = dram.tile([P, 32], F8, kind="ExternalInput")
            oner_in = dram.tile([1, P], F32R, kind="ExternalInput")
            out_sl = dram.tile([TSL, E], F32, kind="ExternalOutput")
            names.update(
                x=xbf_in.name, xsl=xsl_in.name, wqkv=wqkv_in.name, wo=wo_in.name,
                wg=wg_in.name, wl=wl_in.name, wout=wout_in.name,
                cos=cos_in.name, sin=sin_in.name, mask=mask_in.name,
                onec=onec_in.name, oner=oner_in.name, out=out_sl.name)

            # ---- internal DRAM: per-batch AllToAll bounce ----
            a2a_in = [dram.tile([NCORES * HLOC * P, TB], F8, name=f"a2ai{b}")
                      for b in range(B)]
            agy_in = [dram.tile([E, TB], F8, name=f"agyi{b}")
                      for b in range(B)]
            agy_out = [dram.tile([NCORES * E, TB], F8, name=f"agyo{b}",
                                 addr_space="Shared")
                       for b in range(B)]
            a2a_out = [dram.tile([NCORES * HLOC * P, TB], F8,
                                 name=f"a2ao{b}")
                       for b in range(B)]

            RG = [list(range(NCORES))]

            with tc.tile_pool(name="cst", bufs=1) as cst, \
                 tc.tile_pool(name="acc_p", bufs=1) as acc_p, \
                 tc.tile_pool(name="y2T_p", bufs=1) as y2T_p:
                ident = cst.tile([P, P], BF)
                make_identity(nc, ident[:])
                eps_t = cst.tile([P, 1], F32)
                nc.gpsimd.memset(eps_t[:], EPS)
                neg1 = cst.tile([P, 1], F32)
                nc.gpsimd.memset(neg1[:], -1.0)
                c_wo = cst.tile([P, 1], F32)
                nc.gpsimd.memset(c_wo[:], 1.0 / 4096.0)

                # phase-B staging pools opened below the phase-A pools on the
                # pool stack (LIFO) with fresh SBUF addresses, so their DMAs
                # aren't WAR-gated behind attention's last reads.
                phB = ExitStack()
                ot_p = phB.enter_context(tc.tile_pool(name="ot_p", bufs=1))
                wo_p = phB.enter_context(tc.tile_pool(name="wo_p", bufs=1))
                wof0 = wo_p.tile([P, H * 1024], F8, tag="wof0")
                nc.gpsimd.dma_start(
                    out=wof0[:].rearrange("p (hc w) -> p hc w", hc=H),
                    in_=wo_in[:, :, 0:1024].rearrange("hc p w -> p hc w"))
                ot_sb = []

                # ================= phase A: norm+rope+QKV+attention =========
                stgA = ExitStack()
                wqkv_p = stgA.enter_context(tc.tile_pool(name="wqkv_p", bufs=1))
                tabs = stgA.enter_context(tc.tile_pool(name="tabs", bufs=1))
                ytb_p = stgA.enter_context(tc.tile_pool(name="ytb", bufs=2))
                qkvb_p = stgA.enter_context(tc.tile_pool(name="qkvb", bufs=1))
                st_sb = stgA.enter_context(tc.tile_pool(name="st_sb", bufs=2))
                scr_p = stgA.enter_context(tc.tile_pool(name="scr_p", bufs=1))
                st_ps = stgA.enter_context(tc.tile_pool(name="st_ps", bufs=2, space="PSUM"))
                qkv_ps = stgA.enter_context(tc.tile_pool(name="qkv_ps", bufs=1, space="PSUM"))
                at_s_ps = stgA.enter_context(tc.tile_pool(name="at_s_ps", bufs=2, space="PSUM"))
                at_o_ps = stgA.enter_context(tc.tile_pool(name="at_o_ps", bufs=1, space="PSUM"))
                at_db_ps = stgA.enter_context(tc.tile_pool(name="at_db_ps", bufs=1, space="PSUM"))
                at_sb = stgA.enter_context(tc.tile_pool(name="at_sb", bufs=2))

                wqkv_sb = wqkv_p.tile([P, EC * 6 * P], F8)
                nc.sync.dma_start(
                    out=wqkv_sb[:].rearrange("p (e c) -> p e c", e=EC),
                    in_=wqkv_in[:].rearrange("e p c -> p e c"))
                ones_col = tabs.tile([P, 32], F8)
                nc.sync.dma_start(out=ones_col[:], in_=onec_in[:])
                ones_row = tabs.tile([1, P], F32R)
                nc.sync.dma_start(out=ones_row[:], in_=oner_in[:])
                masks = tabs.tile([P, 4 * 512], F8)
                nc.sync.dma_start(
                    out=masks[:].rearrange("p (m w) -> p m w", m=4),
                    in_=mask_in[:].rearrange("m p w -> p m w"))
                # rope tables for own 256 positions -> sbuf [pos%128, (blk, j)]
                cos_all = tabs.tile([P, 2 * 64], BF)
                sin_all = tabs.tile([P, 2 * 64], BF)
                nc.sync.dma_start(
                    out=cos_all[:].rearrange("p (r j) -> p r j", r=2),
                    in_=cos_in[:].rearrange("(r p) j -> p r j", p=P))
                nc.sync.dma_start(
                    out=sin_all[:].rearrange("p (r j) -> p r j", r=2),
                    in_=sin_in[:].rearrange("(r p) j -> p r j", p=P))

                def prep_batch(pb):
                    """norm+rope+transpose own 256 tokens of batch pb,
                    write yT slice to agy_in[pb], then AllGather."""
                    for tt in range(2):
                        row = TB * pb + P * tt
                        x_t = st_sb.tile([P, E], BF, tag="x")
                        nc.sync.dma_start(out=x_t[:], in_=xbf_in[row:row + P, :])
                        y_t = st_sb.tile([P, E], BF, tag="y")
                        t1 = st_sb.tile([P, E], BF, tag="t1")
                        ssq = st_sb.tile([P, 1], F32, tag="ssq")
                        nc.scalar.activation(y_t[:], x_t[:], AF.Square,
                                             accum_out=ssq[:])
                        sq = st_sb.tile([P, 1], F32, tag="sq")
                        nc.scalar.activation(sq[:], ssq[:], AF.Sqrt,
                                             scale=1.0 / E, bias=eps_t[:])
                        s_t = st_sb.tile([P, 1], F32, tag="s")
                        nc.vector.reciprocal(s_t[:], sq[:])
                        xr = x_t[:].rearrange("p (c two h) -> p c two h", two=2, h=64)
                        yr = y_t[:].rearrange("p (c two h) -> p c two h", two=2, h=64)
                        tr = t1[:].rearrange("p (c two h) -> p c two h", two=2, h=64)
                        cb = cos_all[:, 64 * tt:64 * (tt + 1)].rearrange(
                            "p (o j) -> p o j", o=1).broadcast_to([P, EC, 64])
                        sb_ = sin_all[:, 64 * tt:64 * (tt + 1)].rearrange(
                            "p (o j) -> p o j", o=1).broadcast_to([P, EC, 64])
                        nc.vector.scalar_tensor_tensor(
                            out=tr[:, :, 0], in0=xr[:, :, 1], scalar=s_t[:],
                            in1=sb_, op0=OP.mult, op1=OP.mult)
                        nc.vector.scalar_tensor_tensor(
                            out=yr[:, :, 0], in0=xr[:, :, 0], scalar=s_t[:],
                            in1=cb, op0=OP.mult, op1=OP.mult)
                        nc.vector.tensor_tensor(
                            out=yr[:, :, 0], in0=yr[:, :, 0], in1=tr[:, :, 0],
                            op=OP.subtract)
                        nc.vector.scalar_tensor_tensor(
                            out=tr[:, :, 1], in0=xr[:, :, 0], scalar=s_t[:],
                            in1=sb_, op0=OP.mult, op1=OP.mult)
                        nc.vector.scalar_tensor_tensor(
                            out=yr[:, :, 1], in0=xr[:, :, 1], scalar=s_t[:],
                            in1=cb, op0=OP.mult, op1=OP.mult)
                        nc.vector.tensor_tensor(
                            out=yr[:, :, 1], in0=yr[:, :, 1], in1=tr[:, :, 1],
                            op=OP.add)
                        yTo = st_sb.tile([P, EC * P], F8, tag="yTo")
                        yTo_v = yTo[:].rearrange("p (c t) -> p c t", t=P)
                        for gch in range(4):
                            tps = st_ps.tile([P, 512], BF, tag="tps")
                            for c4 in range(4):
                                c = 4 * gch + c4
                                nc.tensor.transpose(
                                    tps[:, P * c4:P * (c4 + 1)],
                                    y_t[:, P * c:P * (c + 1)], ident[:])
                            nc.scalar.activation(
                                yTo_v[:, 4 * gch:4 * gch + 4, :],
                                tps[:].rearrange("p (c t) -> p c t", t=P),
                                AF.Copy, scale=16.0)
                        nc.sync.dma_start(
                            out=agy_in[pb][:].rearrange(
                                "(ec p) t -> p ec t", p=P)[:, :, P * tt:P * (tt + 1)],
                            in_=yTo_v)
                    nc.gpsimd.collective_compute(
                        "AllGather", OP.bypass, replica_groups=RG,
                        ins=[agy_in[pb][:]], outs=[agy_out[pb][:]])

                prep_batch(0)
                prep_batch(1)
                for b in range(B):
                    qt_b = qkvb_p.tile([P, HLOC * S], BF, tag="qt")
                    kt_b = qkvb_p.tile([P, HLOC * S], BF, tag="kt")
                    v_b = qkvb_p.tile([P, HLOC * S], F8, tag="vb")
                    agy_v = agy_out[b][:].rearrange(
                        "(s ec p) t -> s ec p t", s=NCORES, p=P)
                    for tb in range(NTB):    # 256-token blocks (= src core tb)
                        yT = ytb_p.tile([P, EC * ATB], F8, tag="yT")
                        nc.sync.dma_start(
                            out=yT[:].rearrange("p (e t) -> p e t", e=EC),
                            in_=agy_v[tb].rearrange("ec p t -> p ec t"))
                        # QKV matmuls (6 groups of 128 cols)
                        for half in range(2):
                            for g3 in range(3):
                                g = 3 * half + g3
                                pq = qkv_ps.tile([P, ATB], F32, tag=f"pq{g3 % 2}")
                                wv_v = wqkv_sb[:].rearrange(
                                    "p (e c) -> p e c", e=EC)
                                yT_v = yT[:].rearrange("p (e t) -> p e t", e=EC)
                                for ep in range(EC // 2):
                                    nc.tensor.matmul(
                                        pq[:],
                                        wv_v[:, 2 * ep:2 * ep + 2, P * g:P * (g + 1)],
                                        yT_v[:, 2 * ep:2 * ep + 2, :],
                                        start=(ep == 0), stop=(ep == EC // 2 - 1),
                                        perf_mode=DR)
                                h = g % 2
                                col = S * h + ATB * tb
                                if g < 2:      # Q heads (scale folded host-side)
                                    nc.scalar.activation(qt_b[:, col:col + ATB],
                                                         pq[:], AF.Copy,
                                                         scale=1.0 / 8192.0)
                                elif g < 4:    # K heads
                                    nc.scalar.activation(kt_b[:, col:col + ATB],
                                                         pq[:], AF.Copy,
                                                         scale=1.0 / 8192.0)
                                else:          # V heads -> transpose to [tok, DH]
                                    vt_tmp = st_sb.tile([P, ATB], BF, tag="vt")
                                    nc.scalar.activation(vt_tmp[:], pq[:], AF.Copy,
                                                         scale=1.0 / 512.0)
                                    tpv = st_ps.tile([P, 512], BF, tag="tps")
                                    for j in range(ATB // P):
                                        nc.tensor.transpose(
                                            tpv[:, P * j:P * (j + 1)],
                                            vt_tmp[:, P * j:P * (j + 1)], ident[:])
                                    nc.vector.tensor_copy(v_b[:, col:col + ATB],
                                                          tpv[:, 0:ATB])
                    if b + 2 < B:
                        prep_batch(b + 2)
                    # ---- attention for batch b, both local heads ----
                    for h in range(HLOC):
                        hs = S * h
                        for qb in range(4):
                            nk = 4 * qb + 4
                            ps_o = at_o_ps.tile([P, 512], F32, tag="pso")
                            ps_d = at_db_ps.tile([1, 512], F32, tag="db")
                            nkp = nk // 2
                            for ktp in range(nkp):
                                probs = at_sb.tile([P, 1024], F8, tag="probs")
                                for sub in range(2):
                                    kt = 2 * ktp + sub
                                    ps_s = at_s_ps.tile([P, 512], F32, tag="pss")
                                    nc.tensor.matmul(
                                        ps_s[:],
                                        kt_b[:, hs + P * kt: hs + P * (kt + 1)],
                                        qt_b[:, hs + 512 * qb: hs + 512 * (qb + 1)],
                                        start=True, stop=True)
                                    pr = probs[:, 512 * sub:512 * (sub + 1)]
                                    nc.scalar.activation(pr, ps_s[:], AF.Exp,
                                                         bias=neg1[:])
                                    if kt >= 4 * qb:
                                        m = kt - 4 * qb
                                        nc.vector.tensor_tensor(
                                            out=pr, in0=pr,
                                            in1=masks[:, 512 * m:512 * (m + 1)],
                                            op=OP.mult)
                                pr2 = probs[:].rearrange(
                                    "p (two t) -> p two t", two=2)
                                nc.tensor.matmul(
                                    ps_o[:],
                                    v_b[:, hs + 2 * P * ktp: hs + 2 * P * (ktp + 1)].rearrange(
                                        "p (two t) -> p two t", two=2),
                                    pr2,
                                    start=(ktp == 0), stop=(ktp == nkp - 1),
                                    skip_group_check=True, perf_mode=DR)
                                nc.tensor.matmul(
                                    ps_d[:],
                                    ones_col[:].rearrange(
                                        "p (two t) -> p two t", two=2)[:, :, 0:1],
                                    pr2,
                                    start=(ktp == 0), stop=(ktp == nkp - 1),
                                    skip_group_check=True, perf_mode=DR)
                            rd = scr_p.tile([1, 512], F32R, tag="rd")
                            with nc.allow_low_precision(reason="softmax denom"):
                                nc.vector.reciprocal(rd[:], ps_d[:])
                            ps_b = at_db_ps.tile([P, 512], F32, tag="db")
                            nc.tensor.matmul(ps_b[:], ones_row[:], rd[:],
                                             start=True, stop=True)
                            osb = at_sb.tile([P, 512], BF, tag="osb")
                            nc.scalar.copy(osb[:], ps_o[:])
                            ot = at_sb.tile([P, 512], F8, tag="ot")
                            nc.vector.tensor_tensor(out=ot[:], in0=osb[:],
                                                    in1=ps_b[:], op=OP.mult)
                            # tokens 512*qb..512*qb+512 of batch b ->
                            # dest cores 2qb (first 256) and 2qb+1
                            a2a_v = a2a_in[b][:].rearrange(
                                "(d r) t -> d r t", d=NCORES)
                            for m in range(2):
                                nc.sync.dma_start(
                                    out=a2a_v[2 * qb + m, P * h:P * (h + 1), :],
                                    in_=ot[:, TB * m:TB * (m + 1)])
                    nc.gpsimd.collective_compute(
                        "AllToAll", OP.bypass, replica_groups=RG,
                        ins=[a2a_in[b][:]], outs=[a2a_out[b][:]])
                    o8 = ot_p.tile([P, H * TB], F8, name=f"otb8{b}")
                    nc.gpsimd.dma_start(
                        out=o8[:].rearrange("p (hc t) -> p hc t", hc=H),
                        in_=a2a_out[b][:].rearrange("(hc p) t -> p hc t", p=P))
                    ot_sb.append(o8)
                stgA.close()

                # ================= phase B: WO + residual + FFN norm ========
                # fp32 accumulator tiles hold x + attn@wo, later += FFN out.
                acc = []
                for t8 in range(8):
                    a = acc_p.tile([P, E], F32, name=f"acc{t8}")
                    nc.sync.dma_start(out=a[:], in_=xsl_in[P * t8:P * (t8 + 1), :])
                    acc.append(a)
                y2T = y2T_p.tile([P, EC * TSL], BF)

                wo_sb = phB.enter_context(tc.tile_pool(name="wo_sb", bufs=2))
                wo_ps = phB.enter_context(tc.tile_pool(name="wo_ps", bufs=2, space="PSUM"))
                wo_tps = phB.enter_context(tc.tile_pool(name="wo_tps", bufs=2, space="PSUM"))
                wo_p2 = phB.enter_context(tc.tile_pool(name="wo_p2", bufs=1))
                wof1 = wo_p2.tile([P, H * 1024], F8, tag="wof1")
                nc.gpsimd.dma_start(
                    out=wof1[:].rearrange("p (hc w) -> p hc w", hc=H),
                    in_=wo_in[:, :, 1024:2048].rearrange("hc p w -> p hc w"))
                wo_halves = [wof0[:].rearrange("p (hc w) -> p hc w", hc=H),
                             wof1[:].rearrange("p (hc w) -> p hc w", hc=H)]
                for t8 in range(8):
                    b, tt = t8 // 2, t8 % 2
                    ot_v = ot_sb[b][:].rearrange("p (hc t) -> p hc t", hc=H)
                    for ecol in range(4):
                        ps = wo_ps.tile([P, 512], F32, tag="ps")
                        wo_h = wo_halves[ecol // 2]
                        ec2 = ecol % 2
                        for hp in range(H // 2):
                            nc.tensor.matmul(
                                ps[:],
                                ot_v[:, 2 * hp:2 * hp + 2, P * tt:P * (tt + 1)],
                                wo_h[:, 2 * hp:2 * hp + 2,
                                     512 * ec2:512 * (ec2 + 1)],
                                start=(hp == 0), stop=(hp == H // 2 - 1),
                                perf_mode=DR)
                        nc.vector.scalar_tensor_tensor(
                            out=acc[t8][:, 512 * ecol:512 * (ecol + 1)],
                            in0=ps[:], scalar=c_wo[:],
                            in1=acc[t8][:, 512 * ecol:512 * (ecol + 1)],
                            op0=OP.mult, op1=OP.add)
                # FFN rmsnorm + transpose -> y2T [e, tok]
                for t8 in range(8):
                    scr2 = wo_sb.tile([P, E], BF, tag="scr2")
                    ssq2 = wo_sb.tile([P, 1], F32, tag="ssq2")
                    nc.scalar.activation(scr2[:], acc[t8][:], AF.Square,
                                         accum_out=ssq2[:])
                    sq2 = wo_sb.tile([P, 1], F32, tag="sq2")
                    nc.scalar.activation(sq2[:], ssq2[:], AF.Sqrt,
                                         scale=1.0 / E, bias=eps_t[:])
                    s2 = wo_sb.tile([P, 1], F32, tag="s2")
                    nc.vector.reciprocal(s2[:], sq2[:])
                    y2_t = wo_sb.tile([P, E], BF, tag="y2")
                    nc.scalar.activation(y2_t[:], acc[t8][:], AF.Copy,
                                         scale=s2[:])
                    y2T_v = y2T[:].rearrange("p (c t) -> p c t", t=TSL)
                    for gch in range(4):
                        tps = wo_tps.tile([P, 512], BF, tag="tps")
                        for c4 in range(4):
                            c = 4 * gch + c4
                            nc.tensor.transpose(
                                tps[:, P * c4:P * (c4 + 1)],
                                y2_t[:, P * c:P * (c + 1)], ident[:])
                        nc.vector.tensor_copy(
                            y2T_v[:, 4 * gch:4 * gch + 4, P * t8:P * (t8 + 1)],
                            tps[:].rearrange("p (c t) -> p c t", t=P))
                phB.close()

                # ================= phase C: FFN, streamed full weights ======
                phC = ExitStack()
                wf_p = phC.enter_context(tc.tile_pool(name="wf_p", bufs=2))
                hT_p = phC.enter_context(tc.tile_pool(name="hT_p", bufs=1))
                f1_sb = phC.enter_context(tc.tile_pool(name="f1_sb", bufs=2))
                f1_gps = phC.enter_context(tc.tile_pool(name="f1_gps", bufs=2, space="PSUM"))
                f1_lps = phC.enter_context(tc.tile_pool(name="f1_lps", bufs=2, space="PSUM"))
                f2_ps = phC.enter_context(tc.tile_pool(name="f2_ps", bufs=2, space="PSUM"))

                for k in range(FBLK):
                    hT = hT_p.tile([P, FCB * TSL], BF, tag="hT")
                    for fc in range(FCB):
                        wgs = wf_p.tile([P, EC * P], BF, tag="wg")
                        nc.sync.dma_start(
                            out=wgs[:].rearrange("p (e c) -> p e c", e=EC),
                            in_=wg_in[FCB * k + fc].rearrange("e p c -> p e c"))
                        wls = wf_p.tile([P, EC * P], BF, tag="wl")
                        nc.sync.dma_start(
                            out=wls[:].rearrange("p (e c) -> p e c", e=EC),
                            in_=wl_in[FCB * k + fc].rearrange("e p c -> p e c"))
                        for g2 in range(2):   # 512-token groups
                            psg = f1_gps.tile([P, 512], F32, tag="psg")
                            psl = f1_lps.tile([P, 512], F32, tag="psl")
                            for ec in range(EC):
                                mv = y2T[:, TSL * ec + 512 * g2:
                                         TSL * ec + 512 * (g2 + 1)]
                                nc.tensor.matmul(
                                    psg[:], wgs[:, P * ec:P * (ec + 1)], mv,
                                    start=(ec == 0), stop=(ec == EC - 1),
                                    skip_group_check=True)
                                nc.tensor.matmul(
                                    psl[:], wls[:, P * ec:P * (ec + 1)], mv,
                                    start=(ec == 0), stop=(ec == EC - 1),
                                    skip_group_check=True)
                            tmp_g = f1_sb.tile([P, 512], BF, tag="tmpg")
                            nc.scalar.activation(tmp_g[:], psg[:], AF.Gelu)
                            nc.vector.tensor_tensor(
                                out=hT[:, TSL * fc + 512 * g2:
                                       TSL * fc + 512 * (g2 + 1)],
                                in0=tmp_g[:], in1=psl[:], op=OP.mult)
                    # F2: out partial [tok, E] accumulated into acc
                    for ecol in range(4):
                        wos = wf_p.tile([P, FCB * 512], BF, tag="wos")
                        nc.sync.dma_start(
                            out=wos[:].rearrange("p (f w) -> p f w", f=FCB),
                            in_=wout_in[FCB * k:FCB * (k + 1), :,
                                        512 * ecol:512 * (ecol + 1)].rearrange(
                                "f p w -> p f w"))
                        for t8 in range(8):
                            ps2 = f2_ps.tile([P, 512], F32, tag="ps2")
                            for fs in range(FCB):
                                nc.tensor.matmul(
                                    ps2[:],
                                    hT[:, TSL * fs + P * t8: TSL * fs + P * (t8 + 1)],
                                    wos[:, 512 * fs:512 * (fs + 1)],
                                    start=(fs == 0), stop=(fs == FCB - 1))
                            nc.vector.tensor_tensor(
                                out=acc[t8][:, 512 * ecol:512 * (ecol + 1)],
                                in0=ps2[:],
                                in1=acc[t8][:, 512 * ecol:512 * (ecol + 1)],
                                op=OP.add)
                            if k == FBLK - 1 and ecol == 3:
                                nc.sync.dma_start(
                                    out=out_sl[P * t8:P * (t8 + 1), :],
                                    in_=acc[t8][:])
                phC.close()
    nc.compile()
    return nc, names


def _prep_inputs_shared(inputs):
    """Host-side prep of tensors identical on every core."""
    import ml_dtypes
    BF = ml_dtypes.bfloat16
    x = np.ascontiguousarray(
        np.asarray(inputs["inputs"], np.float32).reshape(TOK, E))
    wo = np.asarray(inputs["wo"], np.float32)
    w_gate = np.asarray(inputs["w_gate"], np.float32)
    w_lin = np.asarray(inputs["w_lin"], np.float32)
    w_out = np.asarray(inputs["w_out"], np.float32)
    gamma_attn = np.asarray(inputs["gamma_attn"], np.float32)
    gamma_ffn = np.asarray(inputs["gamma_ffn"], np.float32)
    positions = np.asarray(inputs["positions"])

    F8 = ml_dtypes.float8_e4m3
    wo_r = np.ascontiguousarray((wo.reshape(H, P, E) * 256.0).astype(F8))
    # [E, F] -> [FC, EC, P, P] (fcol-major strips)
    def _gl(w):
        w = (w * gamma_ffn[:, None]).astype(BF)
        return np.ascontiguousarray(
            w.reshape(EC, P, FC, P).transpose(2, 0, 1, 3))
    wg_t = _gl(w_gate)
    wl_t = _gl(w_lin)
    wout_t = np.ascontiguousarray(w_out.reshape(FC, P, E).astype(BF))

    assert np.all(gamma_attn == gamma_attn[0]), \
        "non-uniform gamma_attn needs full-width rope tables"
    half = DH // 2
    inv_freq = (1.0 / (10000.0 ** (np.arange(half, dtype=np.float32) / half))
                ).astype(np.float32)
    ang = positions.astype(np.float32)[:, None] * inv_freq[None, :]
    g0 = float(gamma_attn[0])
    cos = (np.cos(ang) * g0).astype(BF)
    sin = (np.sin(ang) * g0).astype(BF)

    k_i = np.arange(P)[:, None]
    q_i = np.arange(512)[None, :]
    msk = np.stack([(P * m + k_i <= q_i) for m in range(4)]).astype(F8)

    return {
        "wo": wo_r, "wg": wg_t, "wl": wl_t, "wout": wout_t,
        "cos": cos, "sin": sin, "mask": msk,
        "onec": np.ones((P, 32), F8),
        "oner": np.ones((1, P), np.float32),
    }, x


def _prep_inputs_core(inputs, x_f32, r):
    """Per-core tensors: head-sharded QKV weights + owned-token x slice."""
    import ml_dtypes
    F8 = ml_dtypes.float8_e4m3
    wq = np.asarray(inputs["wq"], np.float32) / np.sqrt(np.float32(DH))
    wk = np.asarray(inputs["wk"], np.float32)
    wv = np.asarray(inputs["wv"], np.float32)
    h0 = HLOC * r

    def _slice_qkv(w):   # [E, H, DH] -> [EC, P, HLOC*DH]
        return w[:, h0:h0 + HLOC, :].reshape(EC, P, HLOC * DH)

    wqkv = (np.concatenate([_slice_qkv(wq), _slice_qkv(wk), _slice_qkv(wv)],
                           axis=2) * 512.0).astype(F8)
    # tokens owned by core r: 256 from each batch
    xsl = np.ascontiguousarray(
        x_f32.reshape(B, NCORES, TB, E)[:, r].reshape(TSL, E))
    import ml_dtypes as _md
    xbf = np.ascontiguousarray(xsl.astype(_md.bfloat16))
    return {"wqkv": np.ascontiguousarray(wqkv), "xsl": xsl, "x": xbf}


def _run(inputs, trace=False):
    from concourse.bass_utils import run_bass_kernel_spmd

    if "nc" not in _CACHE:
        _CACHE["nc"], _CACHE["names"] = _build()
    nc, names = _CACHE["nc"], _CACHE["names"]

    shared, x_f32 = _prep_inputs_shared(inputs)
    cos_full = shared.pop("cos")
    sin_full = shared.pop("sin")
    in_maps = []
    for r in range(NCORES):
        prep = dict(shared)
        prep["cos"] = np.ascontiguousarray(cos_full[TB * r:TB * (r + 1)])
        prep["sin"] = np.ascontiguousarray(sin_full[TB * r:TB * (r + 1)])
        prep.update(_prep_inputs_core(inputs, x_f32, r))
        in_maps.append({names[k]: v for k, v in prep.items()})

    res = run_bass_kernel_spmd(nc, in_maps, core_ids=list(range(NCORES)),
                               trace=trace)
    out = np.empty((B, NCORES, TB, E), np.float32)
    for r in range(NCORES):
        out[:, r] = res.results[r][names["out"]].reshape(B, TB, E)
    return out.reshape(B, S, E), res


def kernel(**inputs) -> np.ndarray:
    return _run(inputs)[0]


# revision 35
# speedup vs baseline: 1.1927x; 1.0206x over previous
"""Trainium2 Bass kernel for a dense transformer decoder block.

Tensor-parallel over 8 NeuronCores, bf16 matmuls (fp32 accumulation):
  Phase A: heads sharded (2/core). norm+rope+QKV+causal attention over all
           tokens; per-batch AllToAll redistributes attention output to
           token owners (each core owns 256 tokens of each batch).
  Phase B: WO projection + residual into a persistent fp32 accumulator,
           then FFN rmsnorm -> y2^T.
  Phase C: FFN over the full hidden dim with streamed bf16 weights
           (no collectives); F2 accumulates into the fp32 accumulator.
"""
import sys

if '/opt/trn_rl_repo' not in sys.path:
    sys.path.insert(0, '/opt/trn_rl_repo')

import numpy as np
from contextlib import ExitStack

B, S, E, H, DH, F = 4, 2048, 2048, 16, 128, 8192
P = 128
NCORES = 8
HLOC = H // NCORES          # 2 heads per core
TOK = B * S                 # 8192 tokens
TSL = TOK // NCORES         # 1024 tokens per core (256 from each batch)
TB = TSL // B               # 256 tokens per (core, batch)
EC = E // P                 # 16 embedding chunks
FC = F // P                 # 64 FFN col chunks
FBLK = 4                    # outer F blocks
FCB = FC // FBLK            # 16 col chunks per block
EPS = 1e-5
ATB = 256                   # phase-A token block
NTB = S // ATB              # 8 blocks per batch

_CACHE = {}


def _build():
    import concourse.bacc as bacc
    import concourse.mybir as mybir
    import concourse.tile as tile
    import concourse.tile_utils as tile_utils
    from concourse.masks import make_identity

    tile_utils.max_sbuf_usage = 204 * 1024

    F32 = mybir.dt.float32
    F32R = mybir.dt.float32r
    BF = mybir.dt.bfloat16
    F8 = mybir.dt.float8e4
    DR = mybir.MatmulPerfMode.DoubleRow
    AF = mybir.ActivationFunctionType
    OP = mybir.AluOpType

    nc = bacc.Bacc(None, target_bir_lowering=False)
    names = {}

    with tile.TileContext(nc) as tc:
        with tc.tile_pool(name="dram", bufs=1, space="DRAM") as dram:
            # ---- external inputs ----
            xbf_in = dram.tile([TSL, E], BF, kind="ExternalInput")
            xsl_in = dram.tile([TSL, E], F32, kind="ExternalInput")
            wqkv_in = dram.tile([EC, P, 6 * P], F8, kind="ExternalInput")
            wo_in = dram.tile([H, P, E], F8, kind="ExternalInput")
            wg_in = dram.tile([FC, EC, P, P], BF, kind="ExternalInput")
            wl_in = dram.tile([FC, EC, P, P], BF, kind="ExternalInput")
            wout_in = dram.tile([FC, P, E], BF, kind="ExternalInput")
            cos_in = dram.tile([TB, 64], BF, kind="ExternalInput")
            sin_in = dram.tile([TB, 64], BF, kind="ExternalInput")
            mask_in = dram.tile([4, P, 512], F8, kind="ExternalInput")
            onec_in = dram.tile([P, 32], F8, kind="ExternalInput")
            oner_in = dram.tile([1, P], F32R, kind="ExternalInput")
            out_sl = dram.tile([TSL, E], F32, kind="ExternalOutput")
            names.update(
                x=xbf_in.name, xsl=xsl_in.name, wqkv=wqkv_in.name, wo=wo_in.name,
                wg=wg_in.name, wl=wl_in.name, wout=wout_in.name,
                cos=cos_in.name, sin=sin_in.name, mask=mask_in.name,
                onec=onec_in.name, oner=oner_in.name, out=out_sl.name)

            # ---- internal DRAM: per-batch AllToAll bounce ----
            a2a_in = [dram.tile([NCORES * HLOC * P, TB], F8, name=f"a2ai{b}")
                      for b in range(B)]
            agy_in = [dram.tile([E, TB], F8, name=f"agyi{b}")
                      for b in range(B)]
            agy_out = [dram.tile([NCORES * E, TB], F8, name=f"agyo{b}",
                                 addr_space="Shared")
                       for b in range(B)]
            a2a_out = [dram.tile([NCORES * HLOC * P, TB], F8,
                                 name=f"a2ao{b}")
                       for b in range(B)]

            RG = [list(range(NCORES))]

            with tc.tile_pool(name="cst", bufs=1) as cst, \
                 tc.tile_pool(name="acc_p", bufs=1) as acc_p, \
                 tc.tile_pool(name="y2T_p", bufs=1) as y2T_p:
                ident = cst.tile([P, P], BF)
                make_identity(nc, ident[:])
                eps_t = cst.tile([P, 1], F32)
                nc.gpsimd.memset(eps_t[:], EPS)
                neg1 = cst.tile([P, 1], F32)
                nc.gpsimd.memset(neg1[:], -1.0)
                c_wo = cst.tile([P, 1], F32)
                nc.gpsimd.memset(c_wo[:], 1.0 / 4096.0)

                # phase-B staging pools opened below the phase-A pools on the
                # pool stack (LIFO) with fresh SBUF addresses, so their DMAs
                # aren't WAR-gated behind attention's last reads.
                phB = ExitStack()
                ot_p = phB.enter_context(tc.tile_pool(name="ot_p", bufs=1))
                wo_p = phB.enter_context(tc.tile_pool(name="wo_p", bufs=1))
                wof0 = wo_p.tile([P, H * 1024], F8, tag="wof0")
                nc.gpsimd.dma_start(
                    out=wof0[:].rearrange("p (hc w) -> p hc w", hc=H),
                    in_=wo_in[:, :, 0:1024].rearrange("hc p w -> p hc w"))
                ot_sb = []

                # ================= phase A: norm+rope+QKV+attention =========
                stgA = ExitStack()
                wqkv_p = stgA.enter_context(tc.tile_pool(name="wqkv_p", bufs=1))
                tabs = stgA.enter_context(tc.tile_pool(name="tabs", bufs=1))
                ytb_p = stgA.enter_context(tc.tile_pool(name="ytb", bufs=2))
                qkvb_p = stgA.enter_context(tc.tile_pool(name="qkvb", bufs=1))
                st_sb = stgA.enter_context(tc.tile_pool(name="st_sb", bufs=2))
                scr_p = stgA.enter_context(tc.tile_pool(name="scr_p", bufs=1))
                st_ps = stgA.enter_context(tc.tile_pool(name="st_ps", bufs=2, space="PSUM"))
                qkv_ps = stgA.enter_context(tc.tile_pool(name="qkv_ps", bufs=1, space="PSUM"))
                at_s_ps = stgA.enter_context(tc.tile_pool(name="at_s_ps", bufs=2, space="PSUM"))
                at_o_ps = stgA.enter_context(tc.tile_pool(name="at_o_ps", bufs=1, space="PSUM"))
                at_db_ps = stgA.enter_context(tc.tile_pool(name="at_db_ps", bufs=1, space="PSUM"))
                at_sb = stgA.enter_context(tc.tile_pool(name="at_sb", bufs=2))

                wqkv_sb = wqkv_p.tile([P, EC * 6 * P], F8)
                nc.sync.dma_start(
                    out=wqkv_sb[:].rearrange("p (e c) -> p e c", e=EC),
                    in_=wqkv_in[:].rearrange("e p c -> p e c"))
                ones_col = tabs.tile([P, 32], F8)
                nc.sync.dma_start(out=ones_col[:], in_=onec_in[:])
                ones_row = tabs.tile([1, P], F32R)
                nc.sync.dma_start(out=ones_row[:], in_=oner_in[:])
                masks = tabs.tile([P, 4 * 512], F8)
                nc.sync.dma_start(
                    out=masks[:].rearrange("p (m w) -> p m w", m=4),
                    in_=mask_in[:].rearrange("m p w -> p m w"))
                # rope tables for own 256 positions -> sbuf [pos%128, (blk, j)]
                cos_all = tabs.tile([P, 2 * 64], BF)
                sin_all = tabs.tile([P, 2 * 64], BF)
                nc.sync.dma_start(
                    out=cos_all[:].rearrange("p (r j) -> p r j", r=2),
                    in_=cos_in[:].rearrange("(r p) j -> p r j", p=P))
                nc.sync.dma_start(
                    out=sin_all[:].rearrange("p (r j) -> p r j", r=2),
                    in_=sin_in[:].rearrange("(r p) j -> p r j", p=P))

                def prep_batch(pb):
                    """norm+rope+transpose own 256 tokens of batch pb,
                    write yT slice to agy_in[pb], then AllGather."""
                    for tt in range(2):
                        row = TB * pb + P * tt
                        x_t = st_sb.tile([P, E], BF, tag="x")
                        nc.sync.dma_start(out=x_t[:], in_=xbf_in[row:row + P, :])
                        y_t = st_sb.tile([P, E], BF, tag="y")
                        t1 = st_sb.tile([P, E], BF, tag="t1")
                        ssq = st_sb.tile([P, 1], F32, tag="ssq")
                        nc.scalar.activation(y_t[:], x_t[:], AF.Square,
                                             accum_out=ssq[:])
                        sq = st_sb.tile([P, 1], F32, tag="sq")
                        nc.scalar.activation(sq[:], ssq[:], AF.Sqrt,
                                             scale=1.0 / E, bias=eps_t[:])
                        s_t = st_sb.tile([P, 1], F32, tag="s")
                        nc.vector.reciprocal(s_t[:], sq[:])
                        xr = x_t[:].rearrange("p (c two h) -> p c two h", two=2, h=64)
                        yr = y_t[:].rearrange("p (c two h) -> p c two h", two=2, h=64)
                        tr = t1[:].rearrange("p (c two h) -> p c two h", two=2, h=64)
                        cb = cos_all[:, 64 * tt:64 * (tt + 1)].rearrange(
                            "p (o j) -> p o j", o=1).broadcast_to([P, EC, 64])
                        sb_ = sin_all[:, 64 * tt:64 * (tt + 1)].rearrange(
                            "p (o j) -> p o j", o=1).broadcast_to([P, EC, 64])
                        nc.vector.scalar_tensor_tensor(
                            out=tr[:, :, 0], in0=xr[:, :, 1], scalar=s_t[:],
                            in1=sb_, op0=OP.mult, op1=OP.mult)
                        nc.vector.scalar_tensor_tensor(
                            out=yr[:, :, 0], in0=xr[:, :, 0], scalar=s_t[:],
                            in1=cb, op0=OP.mult, op1=OP.mult)
                        nc.vector.tensor_tensor(
                            out=yr[:, :, 0], in0=yr[:, :, 0], in1=tr[:, :, 0],
                            op=OP.subtract)
                        nc.vector.scalar_tensor_tensor(
                            out=tr[:, :, 1], in0=xr[:, :, 0], scalar=s_t[:],
                            in1=sb_, op0=OP.mult, op1=OP.mult)
                        nc.vector.scalar_tensor_tensor(
                            out=yr[:, :, 1], in0=xr[:, :, 1], scalar=s_t[:],
                            in1=cb, op0=OP.mult, op1=OP.mult)
                        nc.vector.tensor_tensor(
                            out=yr[:, :, 1], in0=yr[:, :, 1], in1=tr[:, :, 1],
                            op=OP.add)
                        yTo = st_sb.tile([P, EC * P], F8, tag="yTo")
                        yTo_v = yTo[:].rearrange("p (c t) -> p c t", t=P)
                        for gch in range(4):
                            tps = st_ps.tile([P, 512], BF, tag="tps")
                            for c4 in range(4):
                                c = 4 * gch + c4
                                nc.tensor.transpose(
                                    tps[:, P * c4:P * (c4 + 1)],
                                    y_t[:, P * c:P * (c + 1)], ident[:])
                            nc.scalar.activation(
                                yTo_v[:, 4 * gch:4 * gch + 4, :],
                                tps[:].rearrange("p (c t) -> p c t", t=P),
                                AF.Copy, scale=16.0)
                        nc.sync.dma_start(
                            out=agy_in[pb][:].rearrange(
                                "(ec p) t -> p ec t", p=P)[:, :, P * tt:P * (tt + 1)],
                            in_=yTo_v)
                    nc.gpsimd.collective_compute(
                        "AllGather", OP.bypass, replica_groups=RG,
                        ins=[agy_in[pb][:]], outs=[agy_out[pb][:]])

                prep_batch(0)
                prep_batch(1)
                for b in range(B):
                    qt_b = qkvb_p.tile([P, HLOC * S], BF, tag="qt")
                    kt_b = qkvb_p.tile([P, HLOC * S], BF, tag="kt")
                    v_b = qkvb_p.tile([P, HLOC * S], F8, tag="vb")
                    agy_v = agy_out[b][:].rearrange(
                        "(s ec p) t -> s ec p t", s=NCORES, p=P)
                    for tb in range(NTB):    # 256-token blocks (= src core tb)
                        yT = ytb_p.tile([P, EC * ATB], F8, tag="yT")
                        nc.sync.dma_start(
                            out=yT[:].rearrange("p (e t) -> p e t", e=EC),
                            in_=agy_v[tb].rearrange("ec p t -> p ec t"))
                        # QKV matmuls (6 groups of 128 cols)
                        for half in range(2):
                            for g3 in range(3):
                                g = 3 * half + g3
                                pq = qkv_ps.tile([P, ATB], F32, tag=f"pq{g3 % 2}")
                                wv_v = wqkv_sb[:].rearrange(
                                    "p (e c) -> p e c", e=EC)
                                yT_v = yT[:].rearrange("p (e t) -> p e t", e=EC)
                                for ep in range(EC // 2):
                                    nc.tensor.matmul(
                                        pq[:],
                                        wv_v[:, 2 * ep:2 * ep + 2, P * g:P * (g + 1)],
                                        yT_v[:, 2 * ep:2 * ep + 2, :],
                                        start=(ep == 0), stop=(ep == EC // 2 - 1),
                                        perf_mode=DR)
                                h = g % 2
                                col = S * h + ATB * tb
                                if g < 2:      # Q heads (scale folded host-side)
                                    nc.scalar.activation(qt_b[:, col:col + ATB],
                                                         pq[:], AF.Copy,
                                                         scale=1.0 / 8192.0)
                                elif g < 4:    # K heads
                                    nc.scalar.activation(kt_b[:, col:col + ATB],
                                                         pq[:], AF.Copy,
                                                         scale=1.0 / 8192.0)
                                else:          # V heads -> transpose to [tok, DH]
                                    vt_tmp = st_sb.tile([P, ATB], BF, tag="vt")
                                    nc.scalar.activation(vt_tmp[:], pq[:], AF.Copy,
                                                         scale=1.0 / 512.0)
                                    tpv = st_ps.tile([P, 512], BF, tag="tps")
                                    for j in range(ATB // P):
                                        nc.tensor.transpose(
                                            tpv[:, P * j:P * (j + 1)],
                                            vt_tmp[:, P * j:P * (j + 1)], ident[:])
                                    nc.vector.tensor_copy(v_b[:, col:col + ATB],
                                                          tpv[:, 0:ATB])
                    if b + 2 < B:
                        prep_batch(b + 2)
                    # ---- attention for batch b, both local heads ----
                    for h in range(HLOC):
                        hs = S * h
                        for qb in range(4):
                            nk = 4 * qb + 4
                            ps_o = at_o_ps.tile([P, 512], F32, tag="pso")
                            ps_d = at_db_ps.tile([1, 512], F32, tag="db")
                            nkp = nk // 2
                            for ktp in range(nkp):
                                probs = at_sb.tile([P, 1024], F8, tag="probs")
                                for sub in range(2):
                                    kt = 2 * ktp + sub
                                    ps_s = at_s_ps.tile([P, 512], F32, tag="pss")
                                    nc.tensor.matmul(
                                        ps_s[:],
                                        kt_b[:, hs + P * kt: hs + P * (kt + 1)],
                                        qt_b[:, hs + 512 * qb: hs + 512 * (qb + 1)],
                                        start=True, stop=True)
                                    pr = probs[:, 512 * sub:512 * (sub + 1)]
                                    nc.scalar.activation(pr, ps_s[:], AF.Exp,
                                                         bias=neg1[:])
                                    if kt >= 4 * qb:
                                        m = kt - 4 * qb
                                        nc.vector.tensor_tensor(
                                            out=pr, in0=pr,
                                            in1=masks[:, 512 * m:512 * (m + 1)],
                                            op=OP.mult)
                                pr2 = probs[:].rearrange(
                                    "p (two t) -> p two t", two=2)
                                nc.tensor.matmul(
                                    ps_o[:],
                                    v_b[:, hs + 2 * P * ktp: hs + 2 * P * (ktp + 1)].rearrange(
                                        "p (two t) -> p two t", two=2),
                                    pr2,
                                    start=(ktp == 0), stop=(ktp == nkp - 1),
                                    skip_group_check=True, perf_mode=DR)
                                nc.tensor.matmul(
                                    ps_d[:],
                                    ones_col[:].rearrange(
                                        "p (two t) -> p two t", two=2)[:, :, 0:1],
                                    pr2,
                                    start=(ktp == 0), stop=(ktp == nkp - 1),
                                    skip_group_check=True, perf_mode=DR)
                            rd = scr_p.tile([1, 512], F32R, tag="rd")
                            with nc.allow_low_precision(reason="softmax denom"):
                                nc.vector.reciprocal(rd[:], ps_d[:])
                            ps_b = at_db_ps.tile([P, 512], F32, tag="db")
                            nc.tensor.matmul(ps_b[:], ones_row[:], rd[:],
                                             start=True, stop=True)
                            osb = at_sb.tile([P, 512], BF, tag="osb")
                            nc.scalar.copy(osb[:], ps_o[:])
                            ot = at_sb.tile([P, 512], F8, tag="ot")
                            nc.vector.tensor_tensor(out=ot[:], in0=osb[:],
                                                    in1=ps_b[:], op=OP.mult)
                            # tokens 512*qb..512*qb+512 of batch b ->
                            # dest cores 2qb (first 256) and 2qb+1
                            a2a_v = a2a_in[b][:].rearrange(
                                "(d r) t -> d r t", d=NCORES)
                            for m in range(2):
                                nc.sync.dma_start(
                                    out=a2a_v[2 * qb + m, P * h:P * (h + 1), :],
                                    in_=ot[:, TB * m:TB * (m + 1)])
                    nc.gpsimd.collective_compute(
                        "AllToAll", OP.bypass, replica_groups=RG,
                        ins=[a2a_in[b][:]], outs=[a2a_out[b][:]])
                    o8 = ot_p.tile([P, H * TB], F8, name=f"otb8{b}")
                    nc.gpsimd.dma_start(
                        out=o8[:].rearrange("p (hc t) -> p hc t", hc=H),
                        in_=a2a_out[b][:].rearrange("(hc p) t -> p hc t", p=P))
                    ot_sb.append(o8)
                stgA.close()

                # ================= phase B: WO + residual + FFN norm ========
                # fp32 accumulator tiles hold x + attn@wo, later += FFN out.
                acc = []
                for t8 in range(8):
                    a = acc_p.tile([P, E], F32, name=f"acc{t8}")
                    nc.sync.dma_start(out=a[:], in_=xsl_in[P * t8:P * (t8 + 1), :])
                    acc.append(a)
                y2T = y2T_p.tile([P, EC * TSL], BF)

                wo_sb = phB.enter_context(tc.tile_pool(name="wo_sb", bufs=2))
                wo_ps = phB.enter_context(tc.tile_pool(name="wo_ps", bufs=2, space="PSUM"))
                wo_tps = phB.enter_context(tc.tile_pool(name="wo_tps", bufs=2, space="PSUM"))
                wo_p2 = phB.enter_context(tc.tile_pool(name="wo_p2", bufs=1))
                wof1 = wo_p2.tile([P, H * 1024], F8, tag="wof1")
                nc.gpsimd.dma_start(
                    out=wof1[:].rearrange("p (hc w) -> p hc w", hc=H),
                    in_=wo_in[:, :, 1024:2048].rearrange("hc p w -> p hc w"))
                wo_halves = [wof0[:].rearrange("p (hc w) -> p hc w", hc=H),
                             wof1[:].rearrange("p (hc w) -> p hc w", hc=H)]
                for t8 in range(8):
                    b, tt = t8 // 2, t8 % 2
                    ot_v = ot_sb[b][:].rearrange("p (hc t) -> p hc t", hc=H)
                    for ecol in range(4):
                        ps = wo_ps.tile([P, 512], F32, tag="ps")
                        wo_h = wo_halves[ecol // 2]
                        ec2 = ecol % 2
                        for hp in range(H // 2):
                            nc.tensor.matmul(
                                ps[:],
                                ot_v[:, 2 * hp:2 * hp + 2, P * tt:P * (tt + 1)],
                                wo_h[:, 2 * hp:2 * hp + 2,
                                     512 * ec2:512 * (ec2 + 1)],
                                start=(hp == 0), stop=(hp == H // 2 - 1),
                                perf_mode=DR)
                        nc.vector.scalar_tensor_tensor(
                            out=acc[t8][:, 512 * ecol:512 * (ecol + 1)],
                            in0=ps[:], scalar=c_wo[:],
                            in1=acc[t8][:, 512 * ecol:512 * (ecol + 1)],
                            op0=OP.mult, op1=OP.add)
                # FFN rmsnorm + transpose -> y2T [e, tok]
                for t8 in range(8):
                    scr2 = wo_sb.tile([P, E], BF, tag="scr2")
                    ssq2 = wo_sb.tile([P, 1], F32, tag="ssq2")
                    nc.scalar.activation(scr2[:], acc[t8][:], AF.Square,
                                         accum_out=ssq2[:])
                    sq2 = wo_sb.tile([P, 1], F32, tag="sq2")
                    nc.scalar.activation(sq2[:], ssq2[:], AF.Sqrt,
                                         scale=1.0 / E, bias=eps_t[:])
                    s2 = wo_sb.tile([P, 1], F32, tag="s2")
                    nc.vector.reciprocal(s2[:], sq2[:])
                    y2_t = wo_sb.tile([P, E], BF, tag="y2")
                    nc.scalar.activation(y2_t[:], acc[t8][:], AF.Copy,
                                         scale=s2[:])
                    y2T_v = y2T[:].rearrange("p (c t) -> p c t", t=TSL)
                    for gch in range(4):
                        tps = wo_tps.tile([P, 512], BF, tag="tps")
                        for c4 in range(4):
                            c = 4 * gch + c4
                            nc.tensor.transpose(
                                tps[:, P * c4:P * (c4 + 1)],
                                y2_t[:, P * c:P * (c + 1)], ident[:])
                        nc.vector.tensor_copy(
                            y2T_v[:, 4 * gch:4 * gch + 4, P * t8:P * (t8 + 1)],
                            tps[:].rearrange("p (c t) -> p c t", t=P))
                phB.close()

                # ================= phase C: FFN, streamed full weights ======
                phC = ExitStack()
                wf_p = phC.enter_context(tc.tile_pool(name="wf_p", bufs=2))
                hT_p = phC.enter_context(tc.tile_pool(name="hT_p", bufs=1))
                f1_sb = phC.enter_context(tc.tile_pool(name="f1_sb", bufs=2))
                f1_gps = phC.enter_context(tc.tile_pool(name="f1_gps", bufs=2, space="PSUM"))
                f1_lps = phC.enter_context(tc.tile_pool(name="f1_lps", bufs=2, space="PSUM"))
                f2_ps = phC.enter_context(tc.tile_pool(name="f2_ps", bufs=2, space="PSUM"))

                for k in range(FBLK):
                    hT = hT_p.tile([P, FCB * TSL], BF, tag="hT")
                    for fc in range(FCB):
                        wgs = wf_p.tile([P, EC * P], BF, tag="wg")
                        nc.sync.dma_start(
                            out=wgs[:].rearrange("p (e c) -> p e c", e=EC),
                            in_=wg_in[FCB * k + fc].rearrange("e p c -> p e c"))
                        wls = wf_p.tile([P, EC * P], BF, tag="wl")
                        nc.sync.dma_start(
                            out=wls[:].rearrange("p (e c) -> p e c", e=EC),
                            in_=wl_in[FCB * k + fc].rearrange("e p c -> p e c"))
                        for g2 in range(2):   # 512-token groups
                            psg = f1_gps.tile([P, 512], F32, tag="psg")
                            psl = f1_lps.tile([P, 512], F32, tag="psl")
                            for ec in range(EC):
                                mv = y2T[:, TSL * ec + 512 * g2:
                                         TSL * ec + 512 * (g2 + 1)]
                                nc.tensor.matmul(
                                    psg[:], wgs[:, P * ec:P * (ec + 1)], mv,
                                    start=(ec == 0), stop=(ec == EC - 1),
                                    skip_group_check=True)
                                nc.tensor.matmul(
                                    psl[:], wls[:, P * ec:P * (ec + 1)], mv,
                                    start=(ec == 0), stop=(ec == EC - 1),
                                    skip_group_check=True)
                            tmp_g = f1_sb.tile([P, 512], BF, tag="tmpg")
                            nc.scalar.activation(tmp_g[:], psg[:], AF.Gelu)
                            nc.vector.tensor_tensor(
                                out=hT[:, TSL * fc + 512 * g2:
                                       TSL * fc + 512 * (g2 + 1)],
                                in0=tmp_g[:], in1=psl[:], op=OP.mult)
                    # F2: out partial [tok, E] accumulated into acc
                    for ecol in range(4):
                        wos = wf_p.tile([P, FCB * 512], BF, tag="wos")
                        nc.sync.dma_start(
                            out=wos[:].rearrange("p (f w) -> p f w", f=FCB),
                            in_=wout_in[FCB * k:FCB * (k + 1), :,
                                        512 * ecol:512 * (ecol + 1)].rearrange(
                                "f p w -> p f w"))
                        for t8 in range(8):
                            ps2 = f2_ps.tile([P, 512], F32, tag="ps2")
                            for fs in range(FCB):
                                nc.tensor.matmul(
                                    ps2[:],
                                    hT[:, TSL * fs + P * t8: TSL * fs + P * (t8 + 1)],
                                    wos[:, 512 * fs:512 * (fs + 1)],
                                    start=(fs == 0), stop=(fs == FCB - 1))
                            nc.vector.tensor_tensor(
                                out=acc[t8][:, 512 * ecol:512 * (ecol + 1)],
                                in0=ps2[:],
                                in1=acc[t8][:, 512 * ecol:512 * (ecol + 1)],
                                op=OP.add)
                            if k == FBLK - 1 and ecol == 3:
                                nc.sync.dma_start(
                                    out=out_sl[P * t8:P * (t8 + 1), :],
                                    in_=acc[t8][:])
                phC.close()
    nc.compile()
    return nc, names


def _prep_inputs_shared(inputs):
    """Host-side prep of tensors identical on every core."""
    import ml_dtypes
    BF = ml_dtypes.bfloat16
    x = np.ascontiguousarray(
        np.asarray(inputs["inputs"], np.float32).reshape(TOK, E))
    wo = np.asarray(inputs["wo"], np.float32)
    w_gate = np.asarray(inputs["w_gate"], np.float32)
    w_lin = np.asarray(inputs["w_lin"], np.float32)
    w_out = np.asarray(inputs["w_out"], np.float32)
    gamma_attn = np.asarray(inputs["gamma_attn"], np.float32)
    gamma_ffn = np.asarray(inputs["gamma_ffn"], np.float32)
    positions = np.asarray(inputs["positions"])

    F8 = ml_dtypes.float8_e4m3
    wo_r = np.ascontiguousarray((wo.reshape(H, P, E) * 256.0).astype(F8))
    # [E, F] -> [FC, EC, P, P] (fcol-major strips)
    def _gl(w):
        w = (w * gamma_ffn[:, None]).astype(BF)
        return np.ascontiguousarray(
            w.reshape(EC, P, FC, P).transpose(2, 0, 1, 3))
    wg_t = _gl(w_gate)
    wl_t = _gl(w_lin)
    wout_t = np.ascontiguousarray(w_out.reshape(FC, P, E).astype(BF))

    assert np.all(gamma_attn == gamma_attn[0]), \
        "non-uniform gamma_attn needs full-width rope tables"
    half = DH // 2
    inv_freq = (1.0 / (10000.0 ** (np.arange(half, dtype=np.float32) / half))
                ).astype(np.float32)
    ang = positions.astype(np.float32)[:, None] * inv_freq[None, :]
    g0 = float(gamma_attn[0])
    cos = (np.cos(ang) * g0).astype(BF)
    sin = (np.sin(ang) * g0).astype(BF)

    k_i = np.arange(P)[:, None]
    q_i = np.arange(512)[None, :]
    msk = np.stack([(P * m + k_i <= q_i) for m in range(4)]).astype(F8)

    return {
        "wo": wo_r, "wg": wg_t, "wl": wl_t, "wout": wout_t,
        "cos": cos, "sin": sin, "mask": msk,
        "onec": np.ones((P, 32), F8),
        "oner": np.ones((1, P), np.float32),
    }, x


def _prep_inputs_core(inputs, x_f32, r):
    """Per-core tensors: head-sharded QKV weights + owned-token x slice."""
    import ml_dtypes
    F8 = ml_dtypes.float8_e4m3
    wq = np.asarray(inputs["wq"], np.float32) / np.sqrt(np.float32(DH))
    wk = np.asarray(inputs["wk"], np.float32)
    wv = np.asarray(inputs["wv"], np.float32)
    h0 = HLOC * r

    def _slice_qkv(w):   # [E, H, DH] -> [EC, P, HLOC*DH]
        return w[:, h0:h0 + HLOC, :].reshape(EC, P, HLOC * DH)

    wqkv = (np.concatenate([_slice_qkv(wq), _slice_qkv(wk), _slice_qkv(wv)],
                           axis=2) * 512.0).astype(F8)
    # tokens owned by core r: 256 from each batch
    xsl = np.ascontiguousarray(
        x_f32.reshape(B, NCORES, TB, E)[:, r].reshape(TSL, E))
    import ml_dtypes as _md
    xbf = np.ascontiguousarray(xsl.astype(_md.bfloat16))
    return {"wqkv": np.ascontiguousarray(wqkv), "xsl": xsl, "x": xbf}


def _run(inputs, trace=False):
    from concourse.bass_utils import run_bass_kernel_spmd

    if "nc" not in _CACHE:
        _CACHE["nc"], _CACHE["names"] = _build()
    nc, names = _CACHE["nc"], _CACHE["names"]

    shared, x_f32 = _prep_inputs_shared(inputs)
    cos_full = shared.pop("cos")
    sin_full = shared.pop("sin")
    in_maps = []
    for r in range(NCORES):
        prep = dict(shared)
        prep["cos"] = np.ascontiguousarray(cos_full[TB * r:TB * (r + 1)])
        prep["sin"] = np.ascontiguousarray(sin_full[TB * r:TB * (r + 1)])
        prep.update(_prep_inputs_core(inputs, x_f32, r))
        in_maps.append({names[k]: v for k, v in prep.items()})

    res = run_bass_kernel_spmd(nc, in_maps, core_ids=list(range(NCORES)),
                               trace=trace)
    out = np.empty((B, NCORES, TB, E), np.float32)
    for r in range(NCORES):
        out[:, r] = res.results[r][names["out"]].reshape(B, TB, E)
    return out.reshape(B, S, E), res


def kernel(**inputs) -> np.ndarray:
    return _run(inputs)[0]
